# revision 53
# baseline (speedup 1.0000x reference)
"""Causal multi-head self-attention (RoPE) Trainium2 kernel.

Model (from the reference nn.Module):
  D_MODEL=1024, NUM_HEADS=16, D_K=64, THETA=10000, BATCH=2, SEQ=2048.
  qkv = x @ w_qkv.T ; q,k get interleaved-pair RoPE; causal softmax(q k^T/8) v;
  out = attn_out @ w_o.T.

Sharding: tensor-parallel over heads. 8 cores x 2 heads each. x is
replicated (transposed on host), per-core w_qkv/w_o head slices. Each core
produces a partial y (full [1024, 4096] f32); host sums partials and
transposes back.

Pipeline per core (all matmul operands f16, PSUM f32):
  - QKV: x resident in SBUF; q/k projected feature-on-partition, RoPE via
    stream_shuffle + fused scalar_tensor_tensor ops (4x DVE mode); V kept
    token-on-partition with an appended ones column per head.
  - Attention: score tiles sT [k=128, q<=512] for both heads in one 2-bank
    PSUM tile; causal mask added on the PE over just the 128-wide diagonal
    band; one exp per k-block on ACT; PV with the exp tile as the
    *stationary* operand -> O accumulates as [q=128, 65*2] (64 dims + the
    softmax denominator per head) using the full 128 output partitions.
  - Normalize: per-partition reciprocal + tensor_scalar, then a DMA
    transpose turns O [q, d] into ocatT [d, q] for the projection.
  - Projection: wo^T . ocatT in 128-row blocks, written straight from PSUM
    to DRAM as f32 by DMA.
  Emission is software-pipelined: QKV chunks of the next batch and
  projection pieces of the previous chunk are interleaved into the
  (ACT-bound) attention block stream so the PE never starves.
"""

import math
import numpy as np
from contextlib import ExitStack

import concourse.bacc as bacc
import concourse.mybir as mybir
import concourse.tile as tile
from concourse.bass_utils import run_bass_kernel_spmd

f32 = mybir.dt.float32
f16 = mybir.dt.float16

D = 1024          # d_model
H = 16            # total heads
DK = 64           # head dim
B = 2
S = 2048
T = B * S         # 4096 tokens
NCORES = 8
HPC = H // NCORES  # heads per core = 2
THETA = 10000.0
NEG = -30000.0     # causal-mask additive constant (exp underflows to 0)

TCH = 512          # token chunk
NTCH = T // TCH    # 8
KCH = 128          # key block
NBLK = T // KCH    # 32

SWAP_MASK = [m ^ 1 for m in range(32)]  # adjacent-pair swap per quadrant

MULT = mybir.AluOpType.mult
ADD = mybir.AluOpType.add

SCHED = {
    "rev_b1": True,        # B1 qi order 3,2,1,0
    "ready": {2: 7, 3: 8, 4: 12, 5: 15, 6: 18, 7: 21},
    "credit0": 0.0,
    "rate": 640.0,
    "proj_lead": 4,
    "act_share": False,    # alternate proj y-copies onto ACT
    "y_queue": "sync",     # which queue triggers y DMAs
    "pool_copies": False,  # y/V PSUM->SBUF copies on gpsimd (Pool)
    "aux_tag": False,      # fillers use their own 1-buf PSUM slot
    "pv_depth": 3,         # blocks of delay between scores/exp and PV
}

DEBUG = False

_PROGRAM = None


def _build_program():
    nc = bacc.Bacc("TRN2", target_bir_lowering=False, debug=False)

    xT = nc.dram_tensor("xT", [D, T], f16, kind="ExternalInput")
    wqkvT = nc.dram_tensor("wqkvT", [D, 3 * 128], f16, kind="ExternalInput")
    woT = nc.dram_tensor("woT", [128, D], f16, kind="ExternalInput")
    crep = nc.dram_tensor("crep", [128, S], f16, kind="ExternalInput")
    ssign = nc.dram_tensor("ssign", [128, S], f16, kind="ExternalInput")
    maskb = nc.dram_tensor("maskb", [128, 2 * 128], f16, kind="ExternalInput")
    identr = nc.dram_tensor("identr", [128, 128], f16, kind="ExternalInput")
    onesd = nc.dram_tensor("onesd", [128, 64], f16, kind="ExternalInput")
    yT = nc.dram_tensor("yT", [D, T], f16, kind="ExternalOutput")
    if DEBUG:
        qTd = nc.dram_tensor("qTd", [128, T], f16, kind="ExternalOutput")
        kTd = nc.dram_tensor("kTd", [128, T], f16, kind="ExternalOutput")
        vd = nc.dram_tensor("vd", [128, NBLK * 130], f16,
                            kind="ExternalOutput")
        ocd = nc.dram_tensor("ocd", [128, T], f16, kind="ExternalOutput")
        ed = nc.dram_tensor("ed", [128, 2 * TCH], f16, kind="ExternalOutput")
        otd = nc.dram_tensor("otd", [128, 4 * 130], f32, kind="ExternalOutput")
        osd = nc.dram_tensor("osd", [128, 4 * 128], f16, kind="ExternalOutput")

    xT_p = xT.rearrange("(n p) t -> p n t", p=128)          # [128, 8, T]
    wq_r = wqkvT.rearrange("(n p) c -> p n c", p=128)       # [128, 8, 384]

    with tile.TileContext(nc) as tc:
        with ExitStack() as ctx:
            singles = ctx.enter_context(tc.tile_pool(name="singles", bufs=1))

            wq_sb = singles.tile([128, 8, 3 * 128], f16)
            wo_sb = singles.tile([128, D], f16)
            crep_sb = singles.tile([128, S], f16)
            ssign_sb = singles.tile([128, S], f16)
            mask_sb = singles.tile([128, 2, 128], f16)
            identr_sb = singles.tile([128, 128], f16)

            # One SP FIFO, ordered by when each tensor is first needed: the
            # DMA_ENGINES device is serial, so arrival order is criticality
            # order. x is resident in SBUF, in 4 groups so early chunks
            # start compute long before the tail groups land. The xcD tail
            # + wo go through the gpsimd SWDGE queue, gated behind xcB by a
            # dummy copy, so the attention-critical transposes on SP slip
            # into the DMA device ahead of them.
            xcA = singles.tile([128, 8, TCH], f16)       # t [0, 512)
            xcB = singles.tile([128, 8, TCH], f16)       # t [512, 1024)
            xcC = singles.tile([128, 8, 2 * TCH], f16)   # t [1024, 2048)
            xcD = singles.tile([128, 8, 4 * TCH], f16)   # t [2048, 4096)
            nc.sync.dma_start(out=wq_sb[:, :, 0:128], in_=wq_r[:, :, 0:128])
            nc.sync.dma_start(out=xcA[:, 0:4, :], in_=xT_p[:, 0:4, 0:512])
            nc.sync.dma_start(out=wq_sb[:, :, 128:256],
                              in_=wq_r[:, :, 128:256])
            nc.sync.dma_start(out=xcA[:, 4:8, :], in_=xT_p[:, 4:8, 0:512])
            nc.sync.dma_start(out=wq_sb[:, :, 256:384],
                              in_=wq_r[:, :, 256:384])
            nc.sync.dma_start(out=crep_sb[:, 0:512], in_=crep[:, 0:512])
            nc.sync.dma_start(out=ssign_sb[:, 0:512], in_=ssign[:, 0:512])
            nc.sync.dma_start(
                out=mask_sb, in_=maskb.rearrange("p (a b) -> p a b", a=2))
            nc.sync.dma_start(out=identr_sb, in_=identr[:, :])
            nc.sync.dma_start(out=xcB, in_=xT_p[:, :, 512:1024])
            nc.sync.dma_start(out=crep_sb[:, 512:2048], in_=crep[:, 512:2048])
            nc.sync.dma_start(out=ssign_sb[:, 512:2048],
                              in_=ssign[:, 512:2048])
            nc.sync.dma_start(out=xcC, in_=xT_p[:, :, 1024:2048])
            nc.sync.dma_start(out=wo_sb, in_=woT[:, :])
            for g in range(4):  # token-quarters: chunk 4+g needs only piece g
                nc.sync.dma_start(
                    out=xcD[:, :, g * 512:(g + 1) * 512],
                    in_=xT_p[:, :, 2048 + g * 512:2048 + (g + 1) * 512])

            def xslice(tch, fo=0, sz=TCH):
                """[128, 8, sz] view of x tokens [tch*512+fo, ...+sz)."""
                t0 = tch * TCH + fo
                if t0 < 512:
                    return xcA[:, :, t0:t0 + sz]
                if t0 < 1024:
                    return xcB[:, :, t0 - 512:t0 - 512 + sz]
                if t0 < 2048:
                    return xcC[:, :, t0 - 1024:t0 - 1024 + sz]
                return xcD[:, :, t0 - 2048:t0 - 2048 + sz]

            qT = singles.tile([128, T], f16)
            kT = singles.tile([128, T], f16)
            # V token-on-partition per 128-token block:
            # [128, blk, head, 65]; col 64 of each head = ones (softmax
            # denominators fall out of the PV matmul's last column).
            vaug = singles.tile([128, NBLK, 2, 65], f16)
            for h in range(2):
                nc.sync.dma_start(out=vaug[:, :, h, 64], in_=onesd[:, 0:NBLK])

            rope = ctx.enter_context(tc.tile_pool(name="rope", bufs=3))
            otr_p = ctx.enter_context(tc.tile_pool(name="otr", bufs=16))
            otrs = {}  # (b, qi, qs) -> contiguous transposed O tile
            eps_p = ctx.enter_context(tc.tile_pool(name="eps", bufs=8))
            osb_p = ctx.enter_context(tc.tile_pool(name="osb", bufs=4))
            rec_p = ctx.enter_context(tc.tile_pool(name="rec", bufs=4))
            y_p = ctx.enter_context(tc.tile_pool(name="yb", bufs=3))
            psum = ctx.enter_context(
                tc.tile_pool(name="ps", bufs=3, space="PSUM"))

            def _filler_tile():
                return psum.tile([128, 2, TCH], f32, tag="sps", bufs=3,
                                 name="aux")

            # ---------------- QKV pieces ----------------
            def rope_emit(ps, dst_sl, s0, act_copy):
                """ps [128,512] f32 PSUM -> RoPE -> dst (f16 SBUF)."""
                psb = rope.tile([128, TCH], f16, tag="psb")
                if act_copy:
                    nc.scalar.activation(
                        out=psb, in_=ps,
                        func=mybir.ActivationFunctionType.Copy)
                else:
                    nc.vector.tensor_copy(out=psb, in_=ps)
                shb = rope.tile([128, TCH], f16, tag="shb")
                nc.vector.stream_shuffle(out=shb, in_=psb, mask=SWAP_MASK)
                t1 = rope.tile([128, TCH], f16, tag="t1")
                nc.vector.tensor_tensor(
                    out=t1, in0=psb, in1=crep_sb[:, s0:s0 + TCH], op=MULT)
                t2 = rope.tile([128, TCH], f16, tag="t2")
                nc.vector.tensor_tensor(
                    out=t2, in0=shb, in1=ssign_sb[:, s0:s0 + TCH], op=MULT)
                nc.vector.tensor_tensor(out=dst_sl, in0=t1, in1=t2, op=ADD)

            def qk_piece(tch, mb, act_copy):
                """Project q (mb=0) or k (mb=1) for token chunk tch + RoPE."""
                t0 = tch * TCH
                s0 = t0 % S
                big = _filler_tile()
                ps = big[:, 0, :]
                xs = xslice(tch)
                for dc in range(8):
                    nc.tensor.matmul(
                        ps, wq_sb[:, dc, mb * 128:(mb + 1) * 128],
                        xs[:, dc, :],
                        start=(dc == 0), stop=(dc == 7),
                        skip_group_check=True)
                dst = qT if mb == 0 else kT
                rope_emit(ps, dst[:, t0:t0 + TCH], s0, act_copy)

            def v_piece(tch, act_copy=False):
                """V for token chunk tch -> vaug blocks (natural layout)."""
                big = _filler_tile()
                for sub in range(4):
                    blk = tch * 4 + sub
                    pv = big[:, 0, sub * 128:(sub + 1) * 128]
                    xs = xslice(tch, fo=sub * KCH, sz=KCH)
                    for dc in range(8):
                        nc.tensor.matmul(
                            pv, xs[:, dc, :],
                            wq_sb[:, dc, 256:384],
                            start=(dc == 0), stop=(dc == 7),
                            skip_group_check=True)
                    for h in range(2):
                        dst = vaug[:, blk, h, 0:64]
                        srch = pv[:, h * 64:(h + 1) * 64]
                        if act_copy:
                            nc.scalar.activation(
                                out=dst, in_=srch,
                                func=mybir.ActivationFunctionType.Copy)
                        elif SCHED["pool_copies"]:
                            nc.gpsimd.tensor_copy(out=dst, in_=srch)
                        else:
                            nc.vector.tensor_copy(out=dst, in_=srch)

            # ---------------- attention ----------------
            def score_exp(b, qi, kj):
                """Scores + mask + exp for block kj; returns the exp tile."""
                toff = b * S
                q0 = toff + qi * TCH
                k0 = toff + kj * KCH
                sub = kj - 4 * qi
                diag = sub >= 0
                o = max(0, KCH * sub)
                pAB = psum.tile([128, 2, TCH], f32, tag="sps", bufs=3)
                nc.tensor.matmul(
                    pAB[:, 0, o:TCH], kT[0:64, k0:k0 + KCH],
                    qT[0:64, q0 + o:q0 + TCH],
                    start=True, stop=not diag, skip_group_check=True)
                nc.tensor.matmul(
                    pAB[:, 1, o:TCH], kT[64:128, k0:k0 + KCH],
                    qT[64:128, q0 + o:q0 + TCH],
                    start=True, stop=not diag, skip_group_check=True)
                if diag:  # additive causal mask, 128-wide band, both heads
                    nc.tensor.matmul(
                        pAB[:, :, o:o + KCH], identr_sb, mask_sb,
                        start=False, stop=True, skip_group_check=True)
                eAB = eps_p.tile([128, 2, TCH], f16, tag="eT")
                nc.scalar.activation(
                    out=eAB[:, :, o:TCH], in_=pAB[:, :, o:TCH],
                    func=mybir.ActivationFunctionType.Exp)
                if DEBUG and (b, qi, kj) == (0, 0, 0):
                    nc.sync.dma_start(
                        out=ed[:, :], in_=eAB.rearrange("p a b -> p (a b)"))
                return eAB

            def pv_block(b, qi, kj, eAB, ot_tiles):
                """PV matmuls for block kj + norms for completed q-subs."""
                blk = b * 16 + kj
                sub = kj - 4 * qi
                for qs in range(max(0, sub), 4):
                    ot = ot_tiles[qs // 2][:, qs % 2, :]
                    for h in range(2):
                        # start=False always: a start=True from the other
                        # head would re-arm the bank and break this head's
                        # open accumulation (verified on HW); tiles are
                        # memset to 0 instead.
                        nc.tensor.matmul(
                            ot[:, h * 65:(h + 1) * 65],
                            eAB[:, h, qs * KCH:(qs + 1) * KCH],
                            vaug[:, blk, h, :],
                            start=False, stop=(kj == 4 * qi + qs),
                            skip_group_check=True)
                if sub >= 0:  # this kj closes q-sub-block `sub`'s bank
                    norm_qsub(b, qi, sub, ot_tiles)

            def norm_qsub(b, qi, qs, ot_tiles):
                ot = ot_tiles[qs // 2][:, qs % 2, :]
                rec = rec_p.tile([128, 2], f32, tag="rc")
                with nc.allow_low_precision(reason="softmax denominators"):
                    nc.vector.reciprocal(out=rec, in_=ot[:, 64::65])
                osb = osb_p.tile([128, 2, 64], f16, tag="ob")
                for h in range(2):
                    nc.vector.tensor_scalar_mul(
                        out=osb[:, h, :],
                        in0=ot[:, h * 65:h * 65 + 64],
                        scalar1=rec[:, h:h + 1])
                if DEBUG and (b, qi) == (0, 0):
                    dsb = osb_p.tile([128, 130], f32, tag="dbg", name="dsb")
                    nc.vector.tensor_copy(out=dsb, in_=ot)
                    nc.sync.dma_start(out=otd[:, qs * 130:(qs + 1) * 130],
                                      in_=dsb)
                    nc.sync.dma_start(
                        out=osd[:, qs * 128:(qs + 1) * 128],
                        in_=osb.rearrange("p a b -> p (a b)"))
                # XBAR transpose needs a CONTIGUOUS destination; strided
                # slices of a big tile produce wrong output on hardware.
                otr = otr_p.tile([128, KCH], f16, tag="otr",
                                 name=f"otr{b}{qi}{qs}")
                nc.sync.dma_start_transpose(out=otr, in_=osb)
                otrs[(b, qi, qs)] = otr

            # ---------------- projection ----------------
            def proj_piece(b, tch, ebp, act_copy=False):
                t0 = b * S + tch * TCH
                pys = _filler_tile()
                for i in range(2):
                    eb = 2 * ebp + i
                    for qs in range(4):
                        nc.tensor.matmul(
                            pys[:, i, qs * KCH:(qs + 1) * KCH],
                            wo_sb[:, eb * 128:(eb + 1) * 128],
                            otrs[(b, tch, qs)],
                            start=True, stop=True, skip_group_check=True)
                ysb = y_p.tile([128, 2, TCH], f16, tag="ysb")
                if act_copy:
                    nc.scalar.activation(
                        out=ysb, in_=pys,
                        func=mybir.ActivationFunctionType.Copy)
                elif SCHED["pool_copies"]:
                    nc.gpsimd.tensor_copy(out=ysb, in_=pys)
                else:
                    nc.vector.tensor_copy(out=ysb, in_=pys)
                eng = nc.gpsimd if SCHED["y_queue"] == "gpsimd" else nc.sync
                # match ysb's (partition, eb-half, col) iteration order on
                # the DRAM side; a flat [256, 512] slice would interleave
                # the output rows pairwise
                yv = yT.rearrange("(a i p) t -> a p i t", i=2, p=128)
                eng.dma_start(out=yv[ebp, :, :, t0:t0 + TCH], in_=ysb)

            # ---------------- schedule ----------------
            # chunks 0,1 up front (ACT does their PSUM->SBUF copies)
            qk_piece(0, 0, True)
            qk_piece(0, 1, True)
            v_piece(0, act_copy=True)
            qk_piece(1, 0, True)
            qk_piece(1, 1, True)
            v_piece(1, act_copy=True)

            # QKV pieces for chunks 2..7 + projection pieces are drained into
            # the 80-block attention stream by credit pacing (total filler PE
            # time / blocks), gated per piece on a readiness block so a
            # not-yet-DMA'd input can't head-of-line-block the in-order PE
            # queue. qi start blocks: (0,*)=0/4/12/24, (1,*)=40/44/52/64.
            QKV_COST = 8 * TCH * 0.4166667
            PROJ_COST = 2 * TCH * 0.4166667
            READY = SCHED["ready"]
            fillers = []  # dicts: cost, ready, fn, chunk?
            for c in range(2, 8):
                for fn in (lambda t=c: qk_piece(t, 0, False),
                           lambda t=c: qk_piece(t, 1, False),
                           lambda t=c: v_piece(t)):
                    fillers.append(
                        {"cost": QKV_COST, "ready": READY[c],
                         "chunk": c, "fn": fn})

            def ensure_chunk(c):
                for f in [f for f in fillers if f.get("chunk") == c]:
                    fillers.remove(f)
                    f["fn"]()

            credit = [SCHED["credit0"]]

            def fill(i):
                while credit[0] > 0:
                    pick = next((f for f in fillers if f["ready"] <= i), None)
                    if pick is None or pick["cost"] > credit[0] + 400:
                        break
                    fillers.remove(pick)
                    pick["fn"]()
                    credit[0] -= pick["cost"]

            # flat block stream, PV delayed one block behind scores/exp so
            # the PE never waits on the current block's exp
            b1o = (3, 2, 1, 0) if SCHED["rev_b1"] else (0, 1, 2, 3)
            stream = [(0, qi, kj) for qi in range(4)
                      for kj in range(4 * qi + 4)]
            stream += [(1, qi, kj) for qi in b1o
                       for kj in range(4 * qi + 4)]
            ots = {}
            pends = []  # [(b, qi, kj, eAB)]

            def flush_pend(limit):
                while len(pends) > limit:
                    pb, pqi, pkj, peAB = pends.pop(0)
                    if (pb, pqi) not in ots:
                        tiles = [
                            psum.tile([128, 2, 130], f32, tag="ot", bufs=2,
                                      name=f"ot{pb}{pqi}{s}")
                            for s in range(2)]
                        for t_ in tiles:
                            nc.vector.memset(t_, 0.0)
                        ots[(pb, pqi)] = tiles

                    pv_block(pb, pqi, pkj, peAB, ots[(pb, pqi)])
                    if pkj == 4 * pqi + 3:
                        last = pb == 1 and pqi <= SCHED.get("act_b1", 0)
                        for ebp in range(4):
                            fillers.append(
                                {"cost": PROJ_COST,
                                 "ready": i_ref[0] + SCHED["proj_lead"],
                                 "fn": lambda b=pb, q=pqi, e=ebp, l=last:
                                 proj_piece(b, q, e, act_copy=(
                                     (SCHED["act_share"] or l)
                                     and e % 2 == 1))})

            i_ref = [0]
            for i, (b, qi, kj) in enumerate(stream):
                i_ref[0] = i
                if kj == 0:
                    if b == 0 and qi >= 2:
                        ensure_chunk(qi)
                    elif b == 1:
                        for c in range(4, 5 + qi):
                            ensure_chunk(c)
                eAB = score_exp(b, qi, kj)
                flush_pend(SCHED["pv_depth"])
                pends.append((b, qi, kj, eAB))
                credit[0] += SCHED["rate"]
                fill(i)
            flush_pend(0)
            for f in fillers:
                f["fn"]()
            if DEBUG:
                nc.sync.dma_start(out=qTd[:, :], in_=qT)
                nc.sync.dma_start(out=kTd[:, :], in_=kT)
                nc.sync.dma_start(
                    out=vd[:, :], in_=vaug.rearrange("p a b c -> p (a b c)"))
                for (db, dqi, dqs), ot_t in otrs.items():
                    qg = db * S + dqi * TCH + dqs * KCH
                    nc.sync.dma_start(out=ocd[:, qg:qg + KCH], in_=ot_t)

    nc.compile()
    return nc


def _host_prep(x, token_positions, w_qkv, w_o):
    """Build per-core input maps."""
    x = np.asarray(x, dtype=np.float32)
    w_qkv = np.asarray(w_qkv, dtype=np.float32)
    w_o = np.asarray(w_o, dtype=np.float32)
    pos = np.asarray(token_positions).astype(np.float64)

    xT = np.ascontiguousarray(x.reshape(T, D).T).astype(np.float16)

    half = DK // 2
    inv_freq = THETA ** (-np.arange(half, dtype=np.float64) / half)  # [32]
    ang = pos[:, None] * inv_freq[None, :]          # [S, 32]
    cos = np.cos(ang).astype(np.float16)            # [S, 32]
    sin = np.sin(ang).astype(np.float16)

    # interleaved pair layout: partition p (within a head's 64) has freq p//2
    cos_rows = np.repeat(cos.T, 2, axis=0)          # [64, S]
    sin_rows = np.repeat(sin.T, 2, axis=0)
    sgn = np.where(np.arange(64) % 2 == 0, -1.0, 1.0).astype(np.float16)
    ssin_rows = sin_rows * sgn[:, None]
    crep = np.vstack([cos_rows, cos_rows])          # [128, 2048]
    ssign = np.vstack([ssin_rows, ssin_rows])

    # strict lower triangle NEG mask for the 128-wide diagonal band, one
    # copy per head: maskb[p, h*128 + j] = NEG if p > j else 0
    jj = np.arange(128)[None, :]
    pp = np.arange(128)[:, None]
    band = np.where(pp > jj, NEG, 0.0).astype(np.float16)
    maskb = np.concatenate([band, band], axis=1)    # [128, 256]

    onesd = np.ones((128, 64), dtype=np.float16)
    identr_np = np.eye(128, dtype=np.float16)

    scale = 1.0 / math.sqrt(DK)
    in_maps = []
    for c in range(NCORES):
        hA, hB = HPC * c, HPC * c + 1
        wq = np.empty((3 * 128, D), dtype=np.float32)
        wq[0:64] = w_qkv[hA * DK:(hA + 1) * DK] * scale
        wq[64:128] = w_qkv[hB * DK:(hB + 1) * DK] * scale
        wq[128:192] = w_qkv[D + hA * DK:D + (hA + 1) * DK]
        wq[192:256] = w_qkv[D + hB * DK:D + (hB + 1) * DK]
        wq[256:320] = w_qkv[2 * D + hA * DK:2 * D + (hA + 1) * DK]
        wq[320:384] = w_qkv[2 * D + hB * DK:2 * D + (hB + 1) * DK]
        wqkvT = np.ascontiguousarray(wq.T).astype(np.float16)

        woTc = np.ascontiguousarray(
            w_o[:, hA * DK:(hB + 1) * DK].T).astype(np.float16)  # [128,1024]

        in_maps.append({
            "xT": xT, "wqkvT": wqkvT, "woT": woTc,
            "crep": crep, "ssign": ssign, "maskb": maskb,
            "onesd": onesd, "identr": identr_np,
        })
    return in_maps


def _get_program():
    global _PROGRAM
    if _PROGRAM is None:
        _PROGRAM = _build_program()
    return _PROGRAM


def run_sharded(in_maps, **kwargs):
    nc = _get_program()
    return run_bass_kernel_spmd(nc, in_maps, core_ids=list(range(NCORES)),
                                **kwargs)


def kernel(x, token_positions, w_qkv, w_o):
    in_maps = _host_prep(x, token_positions, w_qkv, w_o)
    res = run_sharded(in_maps)
    acc = np.zeros((D, T), dtype=np.float64)
    for c in range(NCORES):
        acc += res.results[c]["yT"]
    y = acc.T.astype(np.float32).reshape(B, S, D)
    return y


# revision 54
# speedup vs baseline: 1.0188x; 1.0188x over previous
"""Causal multi-head self-attention (RoPE) Trainium2 kernel.

Model (from the reference nn.Module):
  D_MODEL=1024, NUM_HEADS=16, D_K=64, THETA=10000, BATCH=2, SEQ=2048.
  qkv = x @ w_qkv.T ; q,k get interleaved-pair RoPE; causal softmax(q k^T/8) v;
  out = attn_out @ w_o.T.

Sharding: tensor-parallel over heads. 8 cores x 2 heads each. x is
replicated (transposed on host), per-core w_qkv/w_o head slices. Each core
produces a partial y (full [1024, 4096] f32); host sums partials and
transposes back.

Pipeline per core (all matmul operands f16, PSUM f32):
  - QKV: x resident in SBUF; q/k projected feature-on-partition, RoPE via
    stream_shuffle + fused scalar_tensor_tensor ops (4x DVE mode); V kept
    token-on-partition with an appended ones column per head.
  - Attention: score tiles sT [k=128, q<=512] for both heads in one 2-bank
    PSUM tile; causal mask added on the PE over just the 128-wide diagonal
    band; one exp per k-block on ACT; PV with the exp tile as the
    *stationary* operand -> O accumulates as [q=128, 65*2] (64 dims + the
    softmax denominator per head) using the full 128 output partitions.
  - Normalize: per-partition reciprocal + tensor_scalar, then a DMA
    transpose turns O [q, d] into ocatT [d, q] for the projection.
  - Projection: wo^T . ocatT in 128-row blocks, written straight from PSUM
    to DRAM as f32 by DMA.
  Emission is software-pipelined: QKV chunks of the next batch and
  projection pieces of the previous chunk are interleaved into the
  (ACT-bound) attention block stream so the PE never starves.
"""

import math
import numpy as np
from contextlib import ExitStack

import concourse.bacc as bacc
import concourse.mybir as mybir
import concourse.tile as tile
from concourse.bass_utils import run_bass_kernel_spmd

f32 = mybir.dt.float32
f16 = mybir.dt.float16

D = 1024          # d_model
H = 16            # total heads
DK = 64           # head dim
B = 2
S = 2048
T = B * S         # 4096 tokens
NCORES = 8
HPC = H // NCORES  # heads per core = 2
THETA = 10000.0
NEG = -30000.0     # causal-mask additive constant (exp underflows to 0)

TCH = 512          # token chunk
NTCH = T // TCH    # 8
KCH = 128          # key block
NBLK = T // KCH    # 32

SWAP_MASK = [m ^ 1 for m in range(32)]  # adjacent-pair swap per quadrant

MULT = mybir.AluOpType.mult
ADD = mybir.AluOpType.add

SCHED = {
    "rev_b1": True,        # B1 qi order 3,2,1,0
    "ready": {2: 7, 3: 8, 4: 12, 5: 15, 6: 18, 7: 21},
    "credit0": 0.0,
    "rate": 600.0,
    "proj_lead": 3,
    "act_share": False,    # alternate proj y-copies onto ACT
    "y_queue": "sync",     # which queue triggers y DMAs
    "pool_copies": False,  # y/V PSUM->SBUF copies on gpsimd (Pool)
    "aux_tag": False,      # fillers use their own 1-buf PSUM slot
    "pv_depth": 3,         # blocks of delay between scores/exp and PV
}

DEBUG = False

_PROGRAM = None


def _build_program():
    nc = bacc.Bacc("TRN2", target_bir_lowering=False, debug=False)

    xT = nc.dram_tensor("xT", [D, T], f16, kind="ExternalInput")
    wqkvT = nc.dram_tensor("wqkvT", [D, 3 * 128], f16, kind="ExternalInput")
    woT = nc.dram_tensor("woT", [128, D], f16, kind="ExternalInput")
    crep = nc.dram_tensor("crep", [128, S], f16, kind="ExternalInput")
    ssign = nc.dram_tensor("ssign", [128, S], f16, kind="ExternalInput")
    maskb = nc.dram_tensor("maskb", [128, 2 * 128], f16, kind="ExternalInput")
    identr = nc.dram_tensor("identr", [128, 128], f16, kind="ExternalInput")
    onesd = nc.dram_tensor("onesd", [128, 64], f16, kind="ExternalInput")
    yT = nc.dram_tensor("yT", [D, T], f16, kind="ExternalOutput")
    if DEBUG:
        qTd = nc.dram_tensor("qTd", [128, T], f16, kind="ExternalOutput")
        kTd = nc.dram_tensor("kTd", [128, T], f16, kind="ExternalOutput")
        vd = nc.dram_tensor("vd", [128, NBLK * 130], f16,
                            kind="ExternalOutput")
        ocd = nc.dram_tensor("ocd", [128, T], f16, kind="ExternalOutput")
        ed = nc.dram_tensor("ed", [128, 2 * TCH], f16, kind="ExternalOutput")
        otd = nc.dram_tensor("otd", [128, 4 * 130], f32, kind="ExternalOutput")
        osd = nc.dram_tensor("osd", [128, 4 * 128], f16, kind="ExternalOutput")

    xT_p = xT.rearrange("(n p) t -> p n t", p=128)          # [128, 8, T]
    wq_r = wqkvT.rearrange("(n p) c -> p n c", p=128)       # [128, 8, 384]

    with tile.TileContext(nc) as tc:
        with ExitStack() as ctx:
            singles = ctx.enter_context(tc.tile_pool(name="singles", bufs=1))

            wq_sb = singles.tile([128, 8, 3 * 128], f16)
            wo_sb = singles.tile([128, D], f16)
            crep_sb = singles.tile([128, S], f16)
            ssign_sb = singles.tile([128, S], f16)
            mask_sb = singles.tile([128, 2, 128], f16)
            identr_sb = singles.tile([128, 128], f16)

            # One SP FIFO, ordered by when each tensor is first needed: the
            # DMA_ENGINES device is serial, so arrival order is criticality
            # order. x is resident in SBUF, in 4 groups so early chunks
            # start compute long before the tail groups land. The xcD tail
            # + wo go through the gpsimd SWDGE queue, gated behind xcB by a
            # dummy copy, so the attention-critical transposes on SP slip
            # into the DMA device ahead of them.
            xcA = singles.tile([128, 8, TCH], f16)       # t [0, 512)
            xcB = singles.tile([128, 8, TCH], f16)       # t [512, 1024)
            xcC = singles.tile([128, 8, 2 * TCH], f16)   # t [1024, 2048)
            xcD = singles.tile([128, 8, 4 * TCH], f16)   # t [2048, 4096)
            nc.sync.dma_start(out=wq_sb[:, :, 0:128], in_=wq_r[:, :, 0:128])
            nc.sync.dma_start(out=xcA[:, 0:4, :], in_=xT_p[:, 0:4, 0:512])
            nc.sync.dma_start(out=wq_sb[:, :, 128:256],
                              in_=wq_r[:, :, 128:256])
            nc.sync.dma_start(out=xcA[:, 4:8, :], in_=xT_p[:, 4:8, 0:512])
            nc.sync.dma_start(out=wq_sb[:, :, 256:384],
                              in_=wq_r[:, :, 256:384])
            nc.sync.dma_start(out=crep_sb[:, 0:512], in_=crep[:, 0:512])
            nc.sync.dma_start(out=ssign_sb[:, 0:512], in_=ssign[:, 0:512])
            nc.sync.dma_start(
                out=mask_sb, in_=maskb.rearrange("p (a b) -> p a b", a=2))
            nc.sync.dma_start(out=identr_sb, in_=identr[:, :])
            nc.sync.dma_start(out=xcB, in_=xT_p[:, :, 512:1024])
            nc.sync.dma_start(out=crep_sb[:, 512:2048], in_=crep[:, 512:2048])
            nc.sync.dma_start(out=ssign_sb[:, 512:2048],
                              in_=ssign[:, 512:2048])
            nc.sync.dma_start(out=xcC, in_=xT_p[:, :, 1024:2048])
            nc.sync.dma_start(out=wo_sb, in_=woT[:, :])
            for g in range(4):  # token-quarters: chunk 4+g needs only piece g
                nc.sync.dma_start(
                    out=xcD[:, :, g * 512:(g + 1) * 512],
                    in_=xT_p[:, :, 2048 + g * 512:2048 + (g + 1) * 512])

            def xslice(tch, fo=0, sz=TCH):
                """[128, 8, sz] view of x tokens [tch*512+fo, ...+sz)."""
                t0 = tch * TCH + fo
                if t0 < 512:
                    return xcA[:, :, t0:t0 + sz]
                if t0 < 1024:
                    return xcB[:, :, t0 - 512:t0 - 512 + sz]
                if t0 < 2048:
                    return xcC[:, :, t0 - 1024:t0 - 1024 + sz]
                return xcD[:, :, t0 - 2048:t0 - 2048 + sz]

            qT = singles.tile([128, T], f16)
            kT = singles.tile([128, T], f16)
            # V token-on-partition per 128-token block:
            # [128, blk, head, 65]; col 64 of each head = ones (softmax
            # denominators fall out of the PV matmul's last column).
            vaug = singles.tile([128, NBLK, 2, 65], f16)
            for h in range(2):
                nc.sync.dma_start(out=vaug[:, :, h, 64], in_=onesd[:, 0:NBLK])

            rope = ctx.enter_context(tc.tile_pool(name="rope", bufs=3))
            otr_p = ctx.enter_context(tc.tile_pool(name="otr", bufs=16))
            otrs = {}  # (b, qi, qs) -> contiguous transposed O tile
            eps_p = ctx.enter_context(tc.tile_pool(name="eps", bufs=8))
            osb_p = ctx.enter_context(tc.tile_pool(name="osb", bufs=4))
            rec_p = ctx.enter_context(tc.tile_pool(name="rec", bufs=4))
            y_p = ctx.enter_context(tc.tile_pool(name="yb", bufs=3))
            psum = ctx.enter_context(
                tc.tile_pool(name="ps", bufs=3, space="PSUM"))

            def _filler_tile():
                return psum.tile([128, 2, TCH], f32, tag="sps", bufs=3,
                                 name="aux")

            # ---------------- QKV pieces ----------------
            def rope_emit(ps, dst_sl, s0, act_copy):
                """ps [128,512] f32 PSUM -> RoPE -> dst (f16 SBUF)."""
                psb = rope.tile([128, TCH], f16, tag="psb")
                if act_copy:
                    nc.scalar.activation(
                        out=psb, in_=ps,
                        func=mybir.ActivationFunctionType.Copy)
                else:
                    nc.vector.tensor_copy(out=psb, in_=ps)
                shb = rope.tile([128, TCH], f16, tag="shb")
                nc.vector.stream_shuffle(out=shb, in_=psb, mask=SWAP_MASK)
                t1 = rope.tile([128, TCH], f16, tag="t1")
                nc.vector.tensor_tensor(
                    out=t1, in0=psb, in1=crep_sb[:, s0:s0 + TCH], op=MULT)
                t2 = rope.tile([128, TCH], f16, tag="t2")
                nc.vector.tensor_tensor(
                    out=t2, in0=shb, in1=ssign_sb[:, s0:s0 + TCH], op=MULT)
                nc.vector.tensor_tensor(out=dst_sl, in0=t1, in1=t2, op=ADD)

            def qk_piece(tch, mb, act_copy):
                """Project q (mb=0) or k (mb=1) for token chunk tch + RoPE."""
                t0 = tch * TCH
                s0 = t0 % S
                big = _filler_tile()
                ps = big[:, 0, :]
                xs = xslice(tch)
                for dc in range(8):
                    nc.tensor.matmul(
                        ps, wq_sb[:, dc, mb * 128:(mb + 1) * 128],
                        xs[:, dc, :],
                        start=(dc == 0), stop=(dc == 7),
                        skip_group_check=True)
                dst = qT if mb == 0 else kT
                rope_emit(ps, dst[:, t0:t0 + TCH], s0, act_copy)

            def v_piece(tch, act_copy=False):
                """V for token chunk tch -> vaug blocks (natural layout)."""
                big = _filler_tile()
                for sub in range(4):
                    blk = tch * 4 + sub
                    pv = big[:, 0, sub * 128:(sub + 1) * 128]
                    xs = xslice(tch, fo=sub * KCH, sz=KCH)
                    for dc in range(8):
                        nc.tensor.matmul(
                            pv, xs[:, dc, :],
                            wq_sb[:, dc, 256:384],
                            start=(dc == 0), stop=(dc == 7),
                            skip_group_check=True)
                    for h in range(2):
                        dst = vaug[:, blk, h, 0:64]
                        srch = pv[:, h * 64:(h + 1) * 64]
                        if act_copy:
                            nc.scalar.activation(
                                out=dst, in_=srch,
                                func=mybir.ActivationFunctionType.Copy)
                        elif SCHED["pool_copies"]:
                            nc.gpsimd.tensor_copy(out=dst, in_=srch)
                        else:
                            nc.vector.tensor_copy(out=dst, in_=srch)

            # ---------------- attention ----------------
            def score_exp(b, qi, kj):
                """Scores + mask + exp for block kj; returns the exp tile."""
                toff = b * S
                q0 = toff + qi * TCH
                k0 = toff + kj * KCH
                sub = kj - 4 * qi
                diag = sub >= 0
                o = max(0, KCH * sub)
                pAB = psum.tile([128, 2, TCH], f32, tag="sps", bufs=3)
                nc.tensor.matmul(
                    pAB[:, 0, o:TCH], kT[0:64, k0:k0 + KCH],
                    qT[0:64, q0 + o:q0 + TCH],
                    start=True, stop=not diag, skip_group_check=True)
                nc.tensor.matmul(
                    pAB[:, 1, o:TCH], kT[64:128, k0:k0 + KCH],
                    qT[64:128, q0 + o:q0 + TCH],
                    start=True, stop=not diag, skip_group_check=True)
                if diag:  # additive causal mask, 128-wide band, both heads
                    nc.tensor.matmul(
                        pAB[:, :, o:o + KCH], identr_sb, mask_sb,
                        start=False, stop=True, skip_group_check=True)
                eAB = eps_p.tile([128, 2, TCH], f16, tag="eT")
                nc.scalar.activation(
                    out=eAB[:, :, o:TCH], in_=pAB[:, :, o:TCH],
                    func=mybir.ActivationFunctionType.Exp)
                if DEBUG and (b, qi, kj) == (0, 0, 0):
                    nc.sync.dma_start(
                        out=ed[:, :], in_=eAB.rearrange("p a b -> p (a b)"))
                return eAB

            def pv_block(b, qi, kj, eAB, ot_tiles):
                """PV matmuls for block kj + norms for completed q-subs."""
                blk = b * 16 + kj
                sub = kj - 4 * qi
                for qs in range(max(0, sub), 4):
                    ot = ot_tiles[qs // 2][:, qs % 2, :]
                    for h in range(2):
                        # start=False always: a start=True from the other
                        # head would re-arm the bank and break this head's
                        # open accumulation (verified on HW); tiles are
                        # memset to 0 instead.
                        nc.tensor.matmul(
                            ot[:, h * 65:(h + 1) * 65],
                            eAB[:, h, qs * KCH:(qs + 1) * KCH],
                            vaug[:, blk, h, :],
                            start=False, stop=(kj == 4 * qi + qs),
                            skip_group_check=True)
                if sub >= 0:  # this kj closes q-sub-block `sub`'s bank
                    norm_qsub(b, qi, sub, ot_tiles)

            def norm_qsub(b, qi, qs, ot_tiles):
                ot = ot_tiles[qs // 2][:, qs % 2, :]
                rec = rec_p.tile([128, 2], f32, tag="rc")
                with nc.allow_low_precision(reason="softmax denominators"):
                    nc.vector.reciprocal(out=rec, in_=ot[:, 64::65])
                osb = osb_p.tile([128, 2, 64], f16, tag="ob")
                for h in range(2):
                    nc.vector.tensor_scalar_mul(
                        out=osb[:, h, :],
                        in0=ot[:, h * 65:h * 65 + 64],
                        scalar1=rec[:, h:h + 1])
                if DEBUG and (b, qi) == (0, 0):
                    dsb = osb_p.tile([128, 130], f32, tag="dbg", name="dsb")
                    nc.vector.tensor_copy(out=dsb, in_=ot)
                    nc.sync.dma_start(out=otd[:, qs * 130:(qs + 1) * 130],
                                      in_=dsb)
                    nc.sync.dma_start(
                        out=osd[:, qs * 128:(qs + 1) * 128],
                        in_=osb.rearrange("p a b -> p (a b)"))
                # XBAR transpose needs a CONTIGUOUS destination; strided
                # slices of a big tile produce wrong output on hardware.
                otr = otr_p.tile([128, KCH], f16, tag="otr",
                                 name=f"otr{b}{qi}{qs}")
                nc.sync.dma_start_transpose(out=otr, in_=osb)
                otrs[(b, qi, qs)] = otr

            # ---------------- projection ----------------
            def proj_piece(b, tch, ebp, act_copy=False):
                t0 = b * S + tch * TCH
                pys = _filler_tile()
                for i in range(2):
                    eb = 2 * ebp + i
                    for qs in range(4):
                        nc.tensor.matmul(
                            pys[:, i, qs * KCH:(qs + 1) * KCH],
                            wo_sb[:, eb * 128:(eb + 1) * 128],
                            otrs[(b, tch, qs)],
                            start=True, stop=True, skip_group_check=True)
                ysb = y_p.tile([128, 2, TCH], f16, tag="ysb")
                if act_copy:
                    nc.scalar.activation(
                        out=ysb, in_=pys,
                        func=mybir.ActivationFunctionType.Copy)
                elif SCHED["pool_copies"]:
                    nc.gpsimd.tensor_copy(out=ysb, in_=pys)
                else:
                    nc.vector.tensor_copy(out=ysb, in_=pys)
                eng = nc.gpsimd if SCHED["y_queue"] == "gpsimd" else nc.sync
                # match ysb's (partition, eb-half, col) iteration order on
                # the DRAM side; a flat [256, 512] slice would interleave
                # the output rows pairwise
                yv = yT.rearrange("(a i p) t -> a p i t", i=2, p=128)
                eng.dma_start(out=yv[ebp, :, :, t0:t0 + TCH], in_=ysb)

            # ---------------- schedule ----------------
            # chunks 0,1 up front (ACT does their PSUM->SBUF copies)
            qk_piece(0, 0, True)
            qk_piece(0, 1, True)
            v_piece(0, act_copy=True)
            qk_piece(1, 0, True)
            qk_piece(1, 1, True)
            v_piece(1, act_copy=True)

            # QKV pieces for chunks 2..7 + projection pieces are drained into
            # the 80-block attention stream by credit pacing (total filler PE
            # time / blocks), gated per piece on a readiness block so a
            # not-yet-DMA'd input can't head-of-line-block the in-order PE
            # queue. qi start blocks: (0,*)=0/4/12/24, (1,*)=40/44/52/64.
            QKV_COST = 8 * TCH * 0.4166667
            PROJ_COST = 2 * TCH * 0.4166667
            READY = SCHED["ready"]
            fillers = []  # dicts: cost, ready, fn, chunk?
            for c in range(2, 8):
                for fn in (lambda t=c: qk_piece(t, 0, False),
                           lambda t=c: qk_piece(t, 1, False),
                           lambda t=c: v_piece(t)):
                    fillers.append(
                        {"cost": QKV_COST, "ready": READY[c],
                         "chunk": c, "fn": fn})

            def ensure_chunk(c):
                for f in [f for f in fillers if f.get("chunk") == c]:
                    fillers.remove(f)
                    f["fn"]()

            credit = [SCHED["credit0"]]

            def fill(i):
                while credit[0] > 0:
                    pick = next((f for f in fillers if f["ready"] <= i), None)
                    if pick is None or pick["cost"] > credit[0] + 400:
                        break
                    fillers.remove(pick)
                    pick["fn"]()
                    credit[0] -= pick["cost"]

            # flat block stream, PV delayed one block behind scores/exp so
            # the PE never waits on the current block's exp
            b1o = (3, 2, 1, 0) if SCHED["rev_b1"] else (0, 1, 2, 3)
            stream = [(0, qi, kj) for qi in range(4)
                      for kj in range(4 * qi + 4)]
            stream += [(1, qi, kj) for qi in b1o
                       for kj in range(4 * qi + 4)]
            ots = {}
            pends = []  # [(b, qi, kj, eAB)]

            def flush_pend(limit):
                while len(pends) > limit:
                    pb, pqi, pkj, peAB = pends.pop(0)
                    if (pb, pqi) not in ots:
                        tiles = [
                            psum.tile([128, 2, 130], f32, tag="ot", bufs=2,
                                      name=f"ot{pb}{pqi}{s}")
                            for s in range(2)]
                        for t_ in tiles:
                            nc.vector.memset(t_, 0.0)
                        ots[(pb, pqi)] = tiles

                    pv_block(pb, pqi, pkj, peAB, ots[(pb, pqi)])
                    if pkj == 4 * pqi + 3:
                        last = pb == 1 and pqi <= SCHED.get("act_b1", 0)
                        for ebp in range(4):
                            fillers.append(
                                {"cost": PROJ_COST,
                                 "ready": i_ref[0] + SCHED["proj_lead"],
                                 "fn": lambda b=pb, q=pqi, e=ebp, l=last:
                                 proj_piece(b, q, e, act_copy=(
                                     (SCHED["act_share"] or l)
                                     and e % 2 == 1))})

            i_ref = [0]
            for i, (b, qi, kj) in enumerate(stream):
                i_ref[0] = i
                if kj == 0:
                    if b == 0 and qi >= 2:
                        ensure_chunk(qi)
                    elif b == 1:
                        for c in range(4, 5 + qi):
                            ensure_chunk(c)
                eAB = score_exp(b, qi, kj)
                flush_pend(SCHED["pv_depth"])
                pends.append((b, qi, kj, eAB))
                credit[0] += SCHED["rate"]
                fill(i)
            flush_pend(0)
            for f in fillers:
                f["fn"]()
            if DEBUG:
                nc.sync.dma_start(out=qTd[:, :], in_=qT)
                nc.sync.dma_start(out=kTd[:, :], in_=kT)
                nc.sync.dma_start(
                    out=vd[:, :], in_=vaug.rearrange("p a b c -> p (a b c)"))
                for (db, dqi, dqs), ot_t in otrs.items():
                    qg = db * S + dqi * TCH + dqs * KCH
                    nc.sync.dma_start(out=ocd[:, qg:qg + KCH], in_=ot_t)

    nc.compile()
    return nc


def _host_prep(x, token_positions, w_qkv, w_o):
    """Build per-core input maps."""
    x = np.asarray(x, dtype=np.float32)
    w_qkv = np.asarray(w_qkv, dtype=np.float32)
    w_o = np.asarray(w_o, dtype=np.float32)
    pos = np.asarray(token_positions).astype(np.float64)

    xT = np.ascontiguousarray(x.reshape(T, D).T).astype(np.float16)

    half = DK // 2
    inv_freq = THETA ** (-np.arange(half, dtype=np.float64) / half)  # [32]
    ang = pos[:, None] * inv_freq[None, :]          # [S, 32]
    cos = np.cos(ang).astype(np.float16)            # [S, 32]
    sin = np.sin(ang).astype(np.float16)

    # interleaved pair layout: partition p (within a head's 64) has freq p//2
    cos_rows = np.repeat(cos.T, 2, axis=0)          # [64, S]
    sin_rows = np.repeat(sin.T, 2, axis=0)
    sgn = np.where(np.arange(64) % 2 == 0, -1.0, 1.0).astype(np.float16)
    ssin_rows = sin_rows * sgn[:, None]
    crep = np.vstack([cos_rows, cos_rows])          # [128, 2048]
    ssign = np.vstack([ssin_rows, ssin_rows])

    # strict lower triangle NEG mask for the 128-wide diagonal band, one
    # copy per head: maskb[p, h*128 + j] = NEG if p > j else 0
    jj = np.arange(128)[None, :]
    pp = np.arange(128)[:, None]
    band = np.where(pp > jj, NEG, 0.0).astype(np.float16)
    maskb = np.concatenate([band, band], axis=1)    # [128, 256]

    onesd = np.ones((128, 64), dtype=np.float16)
    identr_np = np.eye(128, dtype=np.float16)

    scale = 1.0 / math.sqrt(DK)
    in_maps = []
    for c in range(NCORES):
        hA, hB = HPC * c, HPC * c + 1
        wq = np.empty((3 * 128, D), dtype=np.float32)
        wq[0:64] = w_qkv[hA * DK:(hA + 1) * DK] * scale
        wq[64:128] = w_qkv[hB * DK:(hB + 1) * DK] * scale
        wq[128:192] = w_qkv[D + hA * DK:D + (hA + 1) * DK]
        wq[192:256] = w_qkv[D + hB * DK:D + (hB + 1) * DK]
        wq[256:320] = w_qkv[2 * D + hA * DK:2 * D + (hA + 1) * DK]
        wq[320:384] = w_qkv[2 * D + hB * DK:2 * D + (hB + 1) * DK]
        wqkvT = np.ascontiguousarray(wq.T).astype(np.float16)

        woTc = np.ascontiguousarray(
            w_o[:, hA * DK:(hB + 1) * DK].T).astype(np.float16)  # [128,1024]

        in_maps.append({
            "xT": xT, "wqkvT": wqkvT, "woT": woTc,
            "crep": crep, "ssign": ssign, "maskb": maskb,
            "onesd": onesd, "identr": identr_np,
        })
    return in_maps


def _get_program():
    global _PROGRAM
    if _PROGRAM is None:
        _PROGRAM = _build_program()
    return _PROGRAM


def run_sharded(in_maps, **kwargs):
    nc = _get_program()
    return run_bass_kernel_spmd(nc, in_maps, core_ids=list(range(NCORES)),
                                **kwargs)


def kernel(x, token_positions, w_qkv, w_o):
    in_maps = _host_prep(x, token_positions, w_qkv, w_o)
    res = run_sharded(in_maps)
    acc = np.zeros((D, T), dtype=np.float64)
    for c in range(NCORES):
        acc += res.results[c]["yT"]
    y = acc.T.astype(np.float32).reshape(B, S, D)
    return y


# revision 59
# speedup vs baseline: 1.0980x; 1.0777x over previous
"""Causal multi-head self-attention (RoPE) Trainium2 kernel.

Model (from the reference nn.Module):
  D_MODEL=1024, NUM_HEADS=16, D_K=64, THETA=10000, BATCH=2, SEQ=2048.
  qkv = x @ w_qkv.T ; q,k get interleaved-pair RoPE; causal softmax(q k^T/8) v;
  out = attn_out @ w_o.T.

Sharding: tensor-parallel over heads. 8 cores x 2 heads each. x is
replicated (transposed on host), per-core w_qkv/w_o head slices. Each core
produces a partial y (full [1024, 4096] f32); host sums partials and
transposes back.

Pipeline per core (all matmul operands f16, PSUM f32):
  - QKV: x resident in SBUF; q/k projected feature-on-partition, RoPE via
    stream_shuffle + fused scalar_tensor_tensor ops (4x DVE mode); V kept
    token-on-partition with an appended ones column per head.
  - Attention: score tiles sT [k=128, q<=512] for both heads in one 2-bank
    PSUM tile; causal mask added on the PE over just the 128-wide diagonal
    band; one exp per k-block on ACT; PV with the exp tile as the
    *stationary* operand -> O accumulates as [q=128, 65*2] (64 dims + the
    softmax denominator per head) using the full 128 output partitions.
  - Normalize: per-partition reciprocal + tensor_scalar, then a DMA
    transpose turns O [q, d] into ocatT [d, q] for the projection.
  - Projection: wo^T . ocatT in 128-row blocks, written straight from PSUM
    to DRAM as f32 by DMA.
  Emission is software-pipelined: QKV chunks of the next batch and
  projection pieces of the previous chunk are interleaved into the
  (ACT-bound) attention block stream so the PE never starves.
"""

import math
import numpy as np
from contextlib import ExitStack

import concourse.bacc as bacc
import concourse.mybir as mybir
import concourse.tile as tile
from concourse.bass_utils import run_bass_kernel_spmd

f32 = mybir.dt.float32
f16 = mybir.dt.float16

D = 1024          # d_model
H = 16            # total heads
DK = 64           # head dim
B = 2
S = 2048
T = B * S         # 4096 tokens
NCORES = 8
HPC = H // NCORES  # heads per core = 2
THETA = 10000.0
NEG = -30000.0     # causal-mask additive constant (exp underflows to 0)

TCH = 512          # token chunk
NTCH = T // TCH    # 8
KCH = 128          # key block
NBLK = T // KCH    # 32

SWAP_MASK = [m ^ 1 for m in range(32)]  # adjacent-pair swap per quadrant

MULT = mybir.AluOpType.mult
ADD = mybir.AluOpType.add

SCHED = {
    "rev_b1": True,        # B1 qi order 3,2,1,0
    "ready": {2: 7, 3: 8, 4: 12, 5: 15, 6: 18, 7: 21},
    "credit0": 0.0,
    "rate": 600.0,
    "proj_lead": 3,
    "act_share": False,    # alternate proj y-copies onto ACT
    "y_queue": "sync",     # which queue triggers y DMAs
    "pool_copies": False,  # y/V PSUM->SBUF copies on gpsimd (Pool)
    "aux_tag": False,      # fillers use their own 1-buf PSUM slot
    "pv_depth": 3,         # blocks of delay between scores/exp and PV
}

DEBUG = False

_PROGRAM = None


def _build_program():
    nc = bacc.Bacc("TRN2", target_bir_lowering=False, debug=False)

    xT = nc.dram_tensor("xT", [D, T], f16, kind="ExternalInput")
    wqkvT = nc.dram_tensor("wqkvT", [D, 3 * 128], f16, kind="ExternalInput")
    woT = nc.dram_tensor("woT", [128, D], f16, kind="ExternalInput")
    crep = nc.dram_tensor("crep", [128, S], f16, kind="ExternalInput")
    ssign = nc.dram_tensor("ssign", [128, S], f16, kind="ExternalInput")
    maskb = nc.dram_tensor("maskb", [128, 2 * 128], f16, kind="ExternalInput")
    identr = nc.dram_tensor("identr", [128, 128], f16, kind="ExternalInput")
    onesd = nc.dram_tensor("onesd", [128, 64], f16, kind="ExternalInput")
    yT = nc.dram_tensor("yT", [D, T], f16, kind="ExternalOutput")
    if DEBUG:
        qTd = nc.dram_tensor("qTd", [128, T], f16, kind="ExternalOutput")
        kTd = nc.dram_tensor("kTd", [128, T], f16, kind="ExternalOutput")
        vd = nc.dram_tensor("vd", [128, NBLK * 130], f16,
                            kind="ExternalOutput")
        ocd = nc.dram_tensor("ocd", [128, T], f16, kind="ExternalOutput")
        ed = nc.dram_tensor("ed", [128, 2 * TCH], f16, kind="ExternalOutput")
        otd = nc.dram_tensor("otd", [128, 4 * 130], f32, kind="ExternalOutput")
        osd = nc.dram_tensor("osd", [128, 4 * 128], f16, kind="ExternalOutput")

    xT_p = xT.rearrange("(n p) t -> p n t", p=128)          # [128, 8, T]
    wq_r = wqkvT.rearrange("(n p) c -> p n c", p=128)       # [128, 8, 384]

    with tile.TileContext(nc) as tc:
        with ExitStack() as ctx:
            singles = ctx.enter_context(tc.tile_pool(name="singles", bufs=1))

            wq_sb = singles.tile([128, 8, 3 * 128], f16)
            wo_sb = singles.tile([128, D], f16)
            crep_sb = singles.tile([128, S], f16)
            ssign_sb = singles.tile([128, S], f16)
            mask_sb = singles.tile([128, 2, 128], f16)
            identr_sb = singles.tile([128, 128], f16)

            # One SP FIFO, ordered by when each tensor is first needed: the
            # DMA_ENGINES device is serial, so arrival order is criticality
            # order. x is resident in SBUF, in 4 groups so early chunks
            # start compute long before the tail groups land. The xcD tail
            # + wo go through the gpsimd SWDGE queue, gated behind xcB by a
            # dummy copy, so the attention-critical transposes on SP slip
            # into the DMA device ahead of them.
            xcA = singles.tile([128, 8, TCH], f16)       # t [0, 512)
            xcB = singles.tile([128, 8, TCH], f16)       # t [512, 1024)
            xcC = singles.tile([128, 8, 2 * TCH], f16)   # t [1024, 2048)
            xcD = singles.tile([128, 8, 4 * TCH], f16)   # t [2048, 4096)
            nc.sync.dma_start(out=wq_sb[:, :, 0:128], in_=wq_r[:, :, 0:128])
            nc.sync.dma_start(out=xcA[:, 0:4, :], in_=xT_p[:, 0:4, 0:512])
            nc.sync.dma_start(out=wq_sb[:, :, 128:256],
                              in_=wq_r[:, :, 128:256])
            nc.sync.dma_start(out=xcA[:, 4:8, :], in_=xT_p[:, 4:8, 0:512])
            nc.sync.dma_start(out=wq_sb[:, :, 256:384],
                              in_=wq_r[:, :, 256:384])
            nc.sync.dma_start(out=crep_sb[:, 0:512], in_=crep[:, 0:512])
            nc.sync.dma_start(out=ssign_sb[:, 0:512], in_=ssign[:, 0:512])
            nc.sync.dma_start(
                out=mask_sb, in_=maskb.rearrange("p (a b) -> p a b", a=2))
            nc.sync.dma_start(out=identr_sb, in_=identr[:, :])
            nc.sync.dma_start(out=xcB, in_=xT_p[:, :, 512:1024])
            nc.sync.dma_start(out=crep_sb[:, 512:2048], in_=crep[:, 512:2048])
            nc.sync.dma_start(out=ssign_sb[:, 512:2048],
                              in_=ssign[:, 512:2048])
            nc.sync.dma_start(out=xcC, in_=xT_p[:, :, 1024:2048])
            nc.sync.dma_start(out=wo_sb, in_=woT[:, :])
            for g in range(4):  # token-quarters: chunk 4+g needs only piece g
                nc.sync.dma_start(
                    out=xcD[:, :, g * 512:(g + 1) * 512],
                    in_=xT_p[:, :, 2048 + g * 512:2048 + (g + 1) * 512])

            def xslice(tch, fo=0, sz=TCH):
                """[128, 8, sz] view of x tokens [tch*512+fo, ...+sz)."""
                t0 = tch * TCH + fo
                if t0 < 512:
                    return xcA[:, :, t0:t0 + sz]
                if t0 < 1024:
                    return xcB[:, :, t0 - 512:t0 - 512 + sz]
                if t0 < 2048:
                    return xcC[:, :, t0 - 1024:t0 - 1024 + sz]
                return xcD[:, :, t0 - 2048:t0 - 2048 + sz]

            qT = singles.tile([128, T], f16)
            kT = singles.tile([128, T], f16)
            # V token-on-partition per 128-token block:
            # [128, blk, head, 65]; col 64 of each head = ones (softmax
            # denominators fall out of the PV matmul's last column).
            vaug = singles.tile([128, NBLK, 2, 65], f16)
            for h in range(2):
                nc.sync.dma_start(out=vaug[:, :, h, 64], in_=onesd[:, 0:NBLK])

            rope = ctx.enter_context(tc.tile_pool(name="rope", bufs=6))
            otr_p = ctx.enter_context(tc.tile_pool(name="otr", bufs=16))
            otrs = {}  # (b, qi, qs) -> contiguous transposed O tile
            eps_p = ctx.enter_context(tc.tile_pool(name="eps", bufs=12))
            osb_p = ctx.enter_context(tc.tile_pool(name="osb", bufs=8))
            rec_p = ctx.enter_context(tc.tile_pool(name="rec", bufs=8))
            y_p = ctx.enter_context(tc.tile_pool(name="yb", bufs=10))
            psum = ctx.enter_context(
                tc.tile_pool(name="ps", bufs=3, space="PSUM"))

            def _filler_tile():
                return psum.tile([128, 2, TCH], f32, tag="sps", bufs=3,
                                 name="aux")

            # ---------------- QKV pieces ----------------
            def rope_emit(ps, dst_sl, s0, act_copy):
                """ps [128,512] f32 PSUM -> RoPE -> dst (f16 SBUF)."""
                psb = rope.tile([128, TCH], f16, tag="psb")
                if act_copy:
                    nc.scalar.activation(
                        out=psb, in_=ps,
                        func=mybir.ActivationFunctionType.Copy)
                else:
                    nc.vector.tensor_copy(out=psb, in_=ps)
                shb = rope.tile([128, TCH], f16, tag="shb")
                nc.vector.stream_shuffle(out=shb, in_=psb, mask=SWAP_MASK)
                t1 = rope.tile([128, TCH], f16, tag="t1")
                nc.vector.tensor_tensor(
                    out=t1, in0=psb, in1=crep_sb[:, s0:s0 + TCH], op=MULT)
                t2 = rope.tile([128, TCH], f16, tag="t2")
                nc.vector.tensor_tensor(
                    out=t2, in0=shb, in1=ssign_sb[:, s0:s0 + TCH], op=MULT)
                nc.vector.tensor_tensor(out=dst_sl, in0=t1, in1=t2, op=ADD)

            def qk_piece(tch, mb, act_copy):
                """Project q (mb=0) or k (mb=1) for token chunk tch + RoPE."""
                t0 = tch * TCH
                s0 = t0 % S
                big = _filler_tile()
                ps = big[:, 0, :]
                xs = xslice(tch)
                for dc in range(8):
                    nc.tensor.matmul(
                        ps, wq_sb[:, dc, mb * 128:(mb + 1) * 128],
                        xs[:, dc, :],
                        start=(dc == 0), stop=(dc == 7),
                        skip_group_check=True)
                dst = qT if mb == 0 else kT
                rope_emit(ps, dst[:, t0:t0 + TCH], s0, act_copy)

            def v_piece(tch, act_copy=False):
                """V for token chunk tch -> vaug blocks (natural layout)."""
                big = _filler_tile()
                for sub in range(4):
                    blk = tch * 4 + sub
                    pv = big[:, 0, sub * 128:(sub + 1) * 128]
                    xs = xslice(tch, fo=sub * KCH, sz=KCH)
                    for dc in range(8):
                        nc.tensor.matmul(
                            pv, xs[:, dc, :],
                            wq_sb[:, dc, 256:384],
                            start=(dc == 0), stop=(dc == 7),
                            skip_group_check=True)
                    for h in range(2):
                        dst = vaug[:, blk, h, 0:64]
                        srch = pv[:, h * 64:(h + 1) * 64]
                        if act_copy:
                            nc.scalar.activation(
                                out=dst, in_=srch,
                                func=mybir.ActivationFunctionType.Copy)
                        else:
                            nc.vector.tensor_copy(out=dst, in_=srch)

            # ---------------- attention ----------------
            def score_exp(b, qi, kj):
                """Scores + mask + exp for block kj; returns the exp tile."""
                toff = b * S
                q0 = toff + qi * TCH
                k0 = toff + kj * KCH
                sub = kj - 4 * qi
                diag = sub >= 0
                o = max(0, KCH * sub)
                pAB = psum.tile([128, 2, TCH], f32, tag="sps", bufs=3)
                nc.tensor.matmul(
                    pAB[:, 0, o:TCH], kT[0:64, k0:k0 + KCH],
                    qT[0:64, q0 + o:q0 + TCH],
                    start=True, stop=not diag, skip_group_check=True)
                nc.tensor.matmul(
                    pAB[:, 1, o:TCH], kT[64:128, k0:k0 + KCH],
                    qT[64:128, q0 + o:q0 + TCH],
                    start=True, stop=not diag, skip_group_check=True)
                if diag:  # additive causal mask, 128-wide band, both heads
                    nc.tensor.matmul(
                        pAB[:, :, o:o + KCH], identr_sb, mask_sb,
                        start=False, stop=True, skip_group_check=True)
                eAB = eps_p.tile([128, 2, TCH], f16, tag="eT")
                nc.scalar.activation(
                    out=eAB[:, :, o:TCH], in_=pAB[:, :, o:TCH],
                    func=mybir.ActivationFunctionType.Exp)
                if DEBUG and (b, qi, kj) == (0, 0, 0):
                    nc.sync.dma_start(
                        out=ed[:, :], in_=eAB.rearrange("p a b -> p (a b)"))
                return eAB

            def pv_block(b, qi, kj, eAB, ot_tiles):
                """PV matmuls for block kj + norms for completed q-subs."""
                blk = b * 16 + kj
                sub = kj - 4 * qi
                for qs in range(max(0, sub), 4):
                    ot = ot_tiles[qs // 2][:, qs % 2, :]
                    for h in range(2):
                        # start=False always: a start=True from the other
                        # head would re-arm the bank and break this head's
                        # open accumulation (verified on HW); tiles are
                        # memset to 0 instead.
                        nc.tensor.matmul(
                            ot[:, h * 65:(h + 1) * 65],
                            eAB[:, h, qs * KCH:(qs + 1) * KCH],
                            vaug[:, blk, h, :],
                            start=False, stop=(kj == 4 * qi + qs),
                            skip_group_check=True)
                if sub >= 0:  # this kj closes q-sub-block `sub`'s bank
                    norm_qsub(b, qi, sub, ot_tiles)

            def norm_qsub(b, qi, qs, ot_tiles):
                ot = ot_tiles[qs // 2][:, qs % 2, :]
                rec = rec_p.tile([128, 2], f32, tag="rc")
                with nc.allow_low_precision(reason="softmax denominators"):
                    nc.vector.reciprocal(out=rec, in_=ot[:, 64::65])
                osb = osb_p.tile([128, 2, 64], f16, tag="ob")
                for h in range(2):
                    nc.vector.tensor_scalar_mul(
                        out=osb[:, h, :],
                        in0=ot[:, h * 65:h * 65 + 64],
                        scalar1=rec[:, h:h + 1])
                if DEBUG and (b, qi) == (0, 0):
                    dsb = osb_p.tile([128, 130], f32, tag="dbg", name="dsb")
                    nc.vector.tensor_copy(out=dsb, in_=ot)
                    nc.sync.dma_start(out=otd[:, qs * 130:(qs + 1) * 130],
                                      in_=dsb)
                    nc.sync.dma_start(
                        out=osd[:, qs * 128:(qs + 1) * 128],
                        in_=osb.rearrange("p a b -> p (a b)"))
                # XBAR transpose needs a CONTIGUOUS destination; strided
                # slices of a big tile produce wrong output on hardware.
                otr = otr_p.tile([128, KCH], f16, tag="otr",
                                 name=f"otr{b}{qi}{qs}")
                nc.sync.dma_start_transpose(out=otr, in_=osb)
                otrs[(b, qi, qs)] = otr

            # ---------------- projection ----------------
            def proj_piece(b, tch, ebp, act_copy=False):
                t0 = b * S + tch * TCH
                pys = _filler_tile()
                for i in range(2):
                    eb = 2 * ebp + i
                    for qs in range(4):
                        nc.tensor.matmul(
                            pys[:, i, qs * KCH:(qs + 1) * KCH],
                            wo_sb[:, eb * 128:(eb + 1) * 128],
                            otrs[(b, tch, qs)],
                            start=True, stop=True, skip_group_check=True)
                ysb = y_p.tile([128, 2, TCH], f16, tag="ysb")
                if act_copy:
                    nc.scalar.activation(
                        out=ysb, in_=pys,
                        func=mybir.ActivationFunctionType.Copy)
                elif SCHED["pool_copies"]:
                    nc.gpsimd.tensor_copy(out=ysb, in_=pys)
                else:
                    nc.vector.tensor_copy(out=ysb, in_=pys)
                eng = nc.gpsimd if SCHED["y_queue"] == "gpsimd" else nc.sync
                # match ysb's (partition, eb-half, col) iteration order on
                # the DRAM side; a flat [256, 512] slice would interleave
                # the output rows pairwise
                yv = yT.rearrange("(a i p) t -> a p i t", i=2, p=128)
                eng.dma_start(out=yv[ebp, :, :, t0:t0 + TCH], in_=ysb)

            # ---------------- schedule ----------------
            # chunks 0,1 up front (ACT does their PSUM->SBUF copies)
            qk_piece(0, 0, True)
            qk_piece(0, 1, True)
            v_piece(0, act_copy=True)
            qk_piece(1, 0, True)
            qk_piece(1, 1, True)
            v_piece(1, act_copy=True)

            # QKV pieces for chunks 2..7 + projection pieces are drained into
            # the 80-block attention stream by credit pacing (total filler PE
            # time / blocks), gated per piece on a readiness block so a
            # not-yet-DMA'd input can't head-of-line-block the in-order PE
            # queue. qi start blocks: (0,*)=0/4/12/24, (1,*)=40/44/52/64.
            QKV_COST = 8 * TCH * 0.4166667
            PROJ_COST = 2 * TCH * 0.4166667
            READY = SCHED["ready"]
            fillers = []  # dicts: cost, ready, fn, chunk?
            for c in range(2, 8):
                for fn in (lambda t=c: qk_piece(t, 0, False),
                           lambda t=c: qk_piece(t, 1, False),
                           lambda t=c: v_piece(t)):
                    fillers.append(
                        {"cost": QKV_COST, "ready": READY[c],
                         "chunk": c, "fn": fn})

            def ensure_chunk(c):
                for f in [f for f in fillers if f.get("chunk") == c]:
                    fillers.remove(f)
                    f["fn"]()

            credit = [SCHED["credit0"]]

            def fill(i):
                while credit[0] > 0:
                    pick = next((f for f in fillers if f["ready"] <= i), None)
                    if pick is None or pick["cost"] > credit[0] + 400:
                        break
                    fillers.remove(pick)
                    pick["fn"]()
                    credit[0] -= pick["cost"]

            # flat block stream, PV delayed one block behind scores/exp so
            # the PE never waits on the current block's exp
            b1o = (3, 2, 1, 0) if SCHED["rev_b1"] else (0, 1, 2, 3)
            stream = [(0, qi, kj) for qi in range(4)
                      for kj in range(4 * qi + 4)]
            stream += [(1, qi, kj) for qi in b1o
                       for kj in range(4 * qi + 4)]
            ots = {}
            pends = []  # [(b, qi, kj, eAB)]

            def flush_pend(limit):
                while len(pends) > limit:
                    pb, pqi, pkj, peAB = pends.pop(0)
                    if (pb, pqi) not in ots:
                        tiles = [
                            psum.tile([128, 2, 130], f32, tag="ot", bufs=2,
                                      name=f"ot{pb}{pqi}{s}")
                            for s in range(2)]
                        for t_ in tiles:
                            nc.vector.memset(t_, 0.0)
                        ots[(pb, pqi)] = tiles

                    pv_block(pb, pqi, pkj, peAB, ots[(pb, pqi)])
                    if pkj == 4 * pqi + 3:
                        last = pb == 1 and pqi <= SCHED.get("act_b1", 0)
                        for ebp in range(4):
                            fillers.append(
                                {"cost": PROJ_COST,
                                 "ready": i_ref[0] + SCHED["proj_lead"],
                                 "fn": lambda b=pb, q=pqi, e=ebp, l=last:
                                 proj_piece(b, q, e, act_copy=(
                                     (SCHED["act_share"] or l)
                                     and e % 2 == 1))})

            i_ref = [0]
            for i, (b, qi, kj) in enumerate(stream):
                i_ref[0] = i
                if kj == 0:
                    if b == 0 and qi >= 2:
                        ensure_chunk(qi)
                    elif b == 1:
                        for c in range(4, 5 + qi):
                            ensure_chunk(c)
                eAB = score_exp(b, qi, kj)
                flush_pend(SCHED["pv_depth"])
                pends.append((b, qi, kj, eAB))
                credit[0] += SCHED["rate"]
                fill(i)
            flush_pend(0)
            for f in fillers:
                f["fn"]()
            if DEBUG:
                nc.sync.dma_start(out=qTd[:, :], in_=qT)
                nc.sync.dma_start(out=kTd[:, :], in_=kT)
                nc.sync.dma_start(
                    out=vd[:, :], in_=vaug.rearrange("p a b c -> p (a b c)"))
                for (db, dqi, dqs), ot_t in otrs.items():
                    qg = db * S + dqi * TCH + dqs * KCH
                    nc.sync.dma_start(out=ocd[:, qg:qg + KCH], in_=ot_t)

    nc.compile()
    return nc


def _host_prep(x, token_positions, w_qkv, w_o):
    """Build per-core input maps."""
    x = np.asarray(x, dtype=np.float32)
    w_qkv = np.asarray(w_qkv, dtype=np.float32)
    w_o = np.asarray(w_o, dtype=np.float32)
    pos = np.asarray(token_positions).astype(np.float64)

    xT = np.ascontiguousarray(x.reshape(T, D).T).astype(np.float16)

    half = DK // 2
    inv_freq = THETA ** (-np.arange(half, dtype=np.float64) / half)  # [32]
    ang = pos[:, None] * inv_freq[None, :]          # [S, 32]
    cos = np.cos(ang).astype(np.float16)            # [S, 32]
    sin = np.sin(ang).astype(np.float16)

    # interleaved pair layout: partition p (within a head's 64) has freq p//2
    cos_rows = np.repeat(cos.T, 2, axis=0)          # [64, S]
    sin_rows = np.repeat(sin.T, 2, axis=0)
    sgn = np.where(np.arange(64) % 2 == 0, -1.0, 1.0).astype(np.float16)
    ssin_rows = sin_rows * sgn[:, None]
    crep = np.vstack([cos_rows, cos_rows])          # [128, 2048]
    ssign = np.vstack([ssin_rows, ssin_rows])

    # strict lower triangle NEG mask for the 128-wide diagonal band, one
    # copy per head: maskb[p, h*128 + j] = NEG if p > j else 0
    jj = np.arange(128)[None, :]
    pp = np.arange(128)[:, None]
    band = np.where(pp > jj, NEG, 0.0).astype(np.float16)
    maskb = np.concatenate([band, band], axis=1)    # [128, 256]

    onesd = np.ones((128, 64), dtype=np.float16)
    identr_np = np.eye(128, dtype=np.float16)

    scale = 1.0 / math.sqrt(DK)
    in_maps = []
    for c in range(NCORES):
        hA, hB = HPC * c, HPC * c + 1
        wq = np.empty((3 * 128, D), dtype=np.float32)
        wq[0:64] = w_qkv[hA * DK:(hA + 1) * DK] * scale
        wq[64:128] = w_qkv[hB * DK:(hB + 1) * DK] * scale
        wq[128:192] = w_qkv[D + hA * DK:D + (hA + 1) * DK]
        wq[192:256] = w_qkv[D + hB * DK:D + (hB + 1) * DK]
        wq[256:320] = w_qkv[2 * D + hA * DK:2 * D + (hA + 1) * DK]
        wq[320:384] = w_qkv[2 * D + hB * DK:2 * D + (hB + 1) * DK]
        wqkvT = np.ascontiguousarray(wq.T).astype(np.float16)

        woTc = np.ascontiguousarray(
            w_o[:, hA * DK:(hB + 1) * DK].T).astype(np.float16)  # [128,1024]

        in_maps.append({
            "xT": xT, "wqkvT": wqkvT, "woT": woTc,
            "crep": crep, "ssign": ssign, "maskb": maskb,
            "onesd": onesd, "identr": identr_np,
        })
    return in_maps


def _get_program():
    global _PROGRAM
    if _PROGRAM is None:
        _PROGRAM = _build_program()
    return _PROGRAM


def run_sharded(in_maps, **kwargs):
    nc = _get_program()
    return run_bass_kernel_spmd(nc, in_maps, core_ids=list(range(NCORES)),
                                **kwargs)


def kernel(x, token_positions, w_qkv, w_o):
    in_maps = _host_prep(x, token_positions, w_qkv, w_o)
    res = run_sharded(in_maps)
    acc = np.zeros((D, T), dtype=np.float64)
    for c in range(NCORES):
        acc += res.results[c]["yT"]
    y = acc.T.astype(np.float32).reshape(B, S, D)
    return y


# revision 61
# speedup vs baseline: 1.1086x; 1.0097x over previous
"""Causal multi-head self-attention (RoPE) Trainium2 kernel.

Model (from the reference nn.Module):
  D_MODEL=1024, NUM_HEADS=16, D_K=64, THETA=10000, BATCH=2, SEQ=2048.
  qkv = x @ w_qkv.T ; q,k get interleaved-pair RoPE; causal softmax(q k^T/8) v;
  out = attn_out @ w_o.T.

Sharding: tensor-parallel over heads. 8 cores x 2 heads each. x is
replicated (transposed on host), per-core w_qkv/w_o head slices. Each core
produces a partial y (full [1024, 4096] f32); host sums partials and
transposes back.

Pipeline per core (all matmul operands f16, PSUM f32):
  - QKV: x resident in SBUF; q/k projected feature-on-partition, RoPE via
    stream_shuffle + fused scalar_tensor_tensor ops (4x DVE mode); V kept
    token-on-partition with an appended ones column per head.
  - Attention: score tiles sT [k=128, q<=512] for both heads in one 2-bank
    PSUM tile; causal mask added on the PE over just the 128-wide diagonal
    band; one exp per k-block on ACT; PV with the exp tile as the
    *stationary* operand -> O accumulates as [q=128, 65*2] (64 dims + the
    softmax denominator per head) using the full 128 output partitions.
  - Normalize: per-partition reciprocal + tensor_scalar, then a DMA
    transpose turns O [q, d] into ocatT [d, q] for the projection.
  - Projection: wo^T . ocatT in 128-row blocks, written straight from PSUM
    to DRAM as f32 by DMA.
  Emission is software-pipelined: QKV chunks of the next batch and
  projection pieces of the previous chunk are interleaved into the
  (ACT-bound) attention block stream so the PE never starves.
"""

import math
import numpy as np
from contextlib import ExitStack

import concourse.bacc as bacc
import concourse.mybir as mybir
import concourse.tile as tile
from concourse.bass_utils import run_bass_kernel_spmd

f32 = mybir.dt.float32
f16 = mybir.dt.float16

D = 1024          # d_model
H = 16            # total heads
DK = 64           # head dim
B = 2
S = 2048
T = B * S         # 4096 tokens
NCORES = 8
HPC = H // NCORES  # heads per core = 2
THETA = 10000.0
NEG = -30000.0     # causal-mask additive constant (exp underflows to 0)

TCH = 512          # token chunk
NTCH = T // TCH    # 8
KCH = 128          # key block
NBLK = T // KCH    # 32

SWAP_MASK = [m ^ 1 for m in range(32)]  # adjacent-pair swap per quadrant

MULT = mybir.AluOpType.mult
ADD = mybir.AluOpType.add

SCHED = {
    "rev_b1": True,        # B1 qi order 3,2,1,0
    "ready": {2: 7, 3: 8, 4: 12, 5: 15, 6: 18, 7: 21},
    "credit0": 0.0,
    "rate": 600.0,
    "proj_lead": 3,
    "act_share": False,    # alternate proj y-copies onto ACT
    "y_queue": "sync",     # which queue triggers y DMAs
    "pool_copies": False,  # y/V PSUM->SBUF copies on gpsimd (Pool)
    "aux_tag": False,      # fillers use their own 1-buf PSUM slot
    "pv_depth": 1,         # blocks of delay between scores/exp and PV
}

DEBUG = False

_PROGRAM = None


def _build_program():
    nc = bacc.Bacc("TRN2", target_bir_lowering=False, debug=False)

    xT = nc.dram_tensor("xT", [D, T], f16, kind="ExternalInput")
    wqkvT = nc.dram_tensor("wqkvT", [D, 3 * 128], f16, kind="ExternalInput")
    woT = nc.dram_tensor("woT", [128, D], f16, kind="ExternalInput")
    crep = nc.dram_tensor("crep", [128, S], f16, kind="ExternalInput")
    ssign = nc.dram_tensor("ssign", [128, S], f16, kind="ExternalInput")
    maskb = nc.dram_tensor("maskb", [128, 2 * 128], f16, kind="ExternalInput")
    identr = nc.dram_tensor("identr", [128, 128], f16, kind="ExternalInput")
    onesd = nc.dram_tensor("onesd", [128, 64], f16, kind="ExternalInput")
    yT = nc.dram_tensor("yT", [D, T], f16, kind="ExternalOutput")
    if DEBUG:
        qTd = nc.dram_tensor("qTd", [128, T], f16, kind="ExternalOutput")
        kTd = nc.dram_tensor("kTd", [128, T], f16, kind="ExternalOutput")
        vd = nc.dram_tensor("vd", [128, NBLK * 130], f16,
                            kind="ExternalOutput")
        ocd = nc.dram_tensor("ocd", [128, T], f16, kind="ExternalOutput")
        ed = nc.dram_tensor("ed", [128, 2 * TCH], f16, kind="ExternalOutput")
        otd = nc.dram_tensor("otd", [128, 4 * 130], f32, kind="ExternalOutput")
        osd = nc.dram_tensor("osd", [128, 4 * 128], f16, kind="ExternalOutput")

    xT_p = xT.rearrange("(n p) t -> p n t", p=128)          # [128, 8, T]
    wq_r = wqkvT.rearrange("(n p) c -> p n c", p=128)       # [128, 8, 384]

    with tile.TileContext(nc) as tc:
        with ExitStack() as ctx:
            singles = ctx.enter_context(tc.tile_pool(name="singles", bufs=1))

            wq_sb = singles.tile([128, 8, 3 * 128], f16)
            wo_sb = singles.tile([128, D], f16)
            crep_sb = singles.tile([128, S], f16)
            ssign_sb = singles.tile([128, S], f16)
            mask_sb = singles.tile([128, 2, 128], f16)
            identr_sb = singles.tile([128, 128], f16)

            # One SP FIFO, ordered by when each tensor is first needed: the
            # DMA_ENGINES device is serial, so arrival order is criticality
            # order. x is resident in SBUF, in 4 groups so early chunks
            # start compute long before the tail groups land. The xcD tail
            # + wo go through the gpsimd SWDGE queue, gated behind xcB by a
            # dummy copy, so the attention-critical transposes on SP slip
            # into the DMA device ahead of them.
            xcA = singles.tile([128, 8, TCH], f16)       # t [0, 512)
            xcB = singles.tile([128, 8, TCH], f16)       # t [512, 1024)
            xcC = singles.tile([128, 8, 2 * TCH], f16)   # t [1024, 2048)
            xcD = singles.tile([128, 8, 4 * TCH], f16)   # t [2048, 4096)
            nc.sync.dma_start(out=wq_sb[:, :, 0:128], in_=wq_r[:, :, 0:128])
            nc.sync.dma_start(out=xcA[:, 0:4, :], in_=xT_p[:, 0:4, 0:512])
            nc.sync.dma_start(out=wq_sb[:, :, 128:256],
                              in_=wq_r[:, :, 128:256])
            nc.sync.dma_start(out=xcA[:, 4:8, :], in_=xT_p[:, 4:8, 0:512])
            nc.sync.dma_start(out=wq_sb[:, :, 256:384],
                              in_=wq_r[:, :, 256:384])
            nc.sync.dma_start(out=crep_sb[:, 0:512], in_=crep[:, 0:512])
            nc.sync.dma_start(out=ssign_sb[:, 0:512], in_=ssign[:, 0:512])
            nc.sync.dma_start(
                out=mask_sb, in_=maskb.rearrange("p (a b) -> p a b", a=2))
            nc.sync.dma_start(out=identr_sb, in_=identr[:, :])
            nc.sync.dma_start(out=xcB, in_=xT_p[:, :, 512:1024])
            nc.sync.dma_start(out=crep_sb[:, 512:2048], in_=crep[:, 512:2048])
            nc.sync.dma_start(out=ssign_sb[:, 512:2048],
                              in_=ssign[:, 512:2048])
            nc.sync.dma_start(out=xcC, in_=xT_p[:, :, 1024:2048])
            nc.sync.dma_start(out=wo_sb, in_=woT[:, :])
            for g in range(4):  # token-quarters: chunk 4+g needs only piece g
                nc.sync.dma_start(
                    out=xcD[:, :, g * 512:(g + 1) * 512],
                    in_=xT_p[:, :, 2048 + g * 512:2048 + (g + 1) * 512])

            def xslice(tch, fo=0, sz=TCH):
                """[128, 8, sz] view of x tokens [tch*512+fo, ...+sz)."""
                t0 = tch * TCH + fo
                if t0 < 512:
                    return xcA[:, :, t0:t0 + sz]
                if t0 < 1024:
                    return xcB[:, :, t0 - 512:t0 - 512 + sz]
                if t0 < 2048:
                    return xcC[:, :, t0 - 1024:t0 - 1024 + sz]
                return xcD[:, :, t0 - 2048:t0 - 2048 + sz]

            qT = singles.tile([128, T], f16)
            kT = singles.tile([128, T], f16)
            # V token-on-partition per 128-token block:
            # [128, blk, head, 65]; col 64 of each head = ones (softmax
            # denominators fall out of the PV matmul's last column).
            vaug = singles.tile([128, NBLK, 2, 65], f16)
            for h in range(2):
                nc.sync.dma_start(out=vaug[:, :, h, 64], in_=onesd[:, 0:NBLK])

            rope = ctx.enter_context(tc.tile_pool(name="rope", bufs=8))
            otr_p = ctx.enter_context(tc.tile_pool(name="otr", bufs=16))
            otrs = {}  # (b, qi, qs) -> contiguous transposed O tile
            eps_p = ctx.enter_context(tc.tile_pool(name="eps", bufs=14))
            osb_p = ctx.enter_context(tc.tile_pool(name="osb", bufs=8))
            rec_p = ctx.enter_context(tc.tile_pool(name="rec", bufs=8))
            y_p = ctx.enter_context(tc.tile_pool(name="yb", bufs=12))
            psum = ctx.enter_context(
                tc.tile_pool(name="ps", bufs=3, space="PSUM"))

            def _filler_tile():
                return psum.tile([128, 2, TCH], f32, tag="sps", bufs=3,
                                 name="aux")

            # ---------------- QKV pieces ----------------
            def rope_emit(ps, dst_sl, s0, act_copy):
                """ps [128,512] f32 PSUM -> RoPE -> dst (f16 SBUF)."""
                psb = rope.tile([128, TCH], f16, tag="psb")
                if act_copy:
                    nc.scalar.activation(
                        out=psb, in_=ps,
                        func=mybir.ActivationFunctionType.Copy)
                else:
                    nc.vector.tensor_copy(out=psb, in_=ps)
                shb = rope.tile([128, TCH], f16, tag="shb")
                nc.vector.stream_shuffle(out=shb, in_=psb, mask=SWAP_MASK)
                t1 = rope.tile([128, TCH], f16, tag="t1")
                nc.vector.tensor_tensor(
                    out=t1, in0=psb, in1=crep_sb[:, s0:s0 + TCH], op=MULT)
                t2 = rope.tile([128, TCH], f16, tag="t2")
                nc.vector.tensor_tensor(
                    out=t2, in0=shb, in1=ssign_sb[:, s0:s0 + TCH], op=MULT)
                nc.vector.tensor_tensor(out=dst_sl, in0=t1, in1=t2, op=ADD)

            def qk_piece(tch, mb, act_copy):
                """Project q (mb=0) or k (mb=1) for token chunk tch + RoPE."""
                t0 = tch * TCH
                s0 = t0 % S
                big = _filler_tile()
                ps = big[:, 0, :]
                xs = xslice(tch)
                for dc in range(8):
                    nc.tensor.matmul(
                        ps, wq_sb[:, dc, mb * 128:(mb + 1) * 128],
                        xs[:, dc, :],
                        start=(dc == 0), stop=(dc == 7),
                        skip_group_check=True)
                dst = qT if mb == 0 else kT
                rope_emit(ps, dst[:, t0:t0 + TCH], s0, act_copy)

            def v_piece(tch, act_copy=False):
                """V for token chunk tch -> vaug blocks (natural layout)."""
                big = _filler_tile()
                for sub in range(4):
                    blk = tch * 4 + sub
                    pv = big[:, 0, sub * 128:(sub + 1) * 128]
                    xs = xslice(tch, fo=sub * KCH, sz=KCH)
                    for dc in range(8):
                        nc.tensor.matmul(
                            pv, xs[:, dc, :],
                            wq_sb[:, dc, 256:384],
                            start=(dc == 0), stop=(dc == 7),
                            skip_group_check=True)
                    for h in range(2):
                        dst = vaug[:, blk, h, 0:64]
                        srch = pv[:, h * 64:(h + 1) * 64]
                        if act_copy:
                            nc.scalar.activation(
                                out=dst, in_=srch,
                                func=mybir.ActivationFunctionType.Copy)
                        else:
                            nc.vector.tensor_copy(out=dst, in_=srch)

            # ---------------- attention ----------------
            def score_exp(b, qi, kj):
                """Scores + mask + exp for block kj; returns the exp tile."""
                toff = b * S
                q0 = toff + qi * TCH
                k0 = toff + kj * KCH
                sub = kj - 4 * qi
                diag = sub >= 0
                o = max(0, KCH * sub)
                pAB = psum.tile([128, 2, TCH], f32, tag="sps", bufs=3)
                nc.tensor.matmul(
                    pAB[:, 0, o:TCH], kT[0:64, k0:k0 + KCH],
                    qT[0:64, q0 + o:q0 + TCH],
                    start=True, stop=not diag, skip_group_check=True)
                nc.tensor.matmul(
                    pAB[:, 1, o:TCH], kT[64:128, k0:k0 + KCH],
                    qT[64:128, q0 + o:q0 + TCH],
                    start=True, stop=not diag, skip_group_check=True)
                if diag:  # additive causal mask, 128-wide band, both heads
                    nc.tensor.matmul(
                        pAB[:, :, o:o + KCH], identr_sb, mask_sb,
                        start=False, stop=True, skip_group_check=True)
                eAB = eps_p.tile([128, 2, TCH], f16, tag="eT")
                nc.scalar.activation(
                    out=eAB[:, :, o:TCH], in_=pAB[:, :, o:TCH],
                    func=mybir.ActivationFunctionType.Exp)
                if DEBUG and (b, qi, kj) == (0, 0, 0):
                    nc.sync.dma_start(
                        out=ed[:, :], in_=eAB.rearrange("p a b -> p (a b)"))
                return eAB

            def pv_block(b, qi, kj, eAB, ot_tiles):
                """PV matmuls for block kj + norms for completed q-subs."""
                blk = b * 16 + kj
                sub = kj - 4 * qi
                for qs in range(max(0, sub), 4):
                    ot = ot_tiles[qs // 2][:, qs % 2, :]
                    for h in range(2):
                        # start=False always: a start=True from the other
                        # head would re-arm the bank and break this head's
                        # open accumulation (verified on HW); tiles are
                        # memset to 0 instead.
                        nc.tensor.matmul(
                            ot[:, h * 65:(h + 1) * 65],
                            eAB[:, h, qs * KCH:(qs + 1) * KCH],
                            vaug[:, blk, h, :],
                            start=False, stop=(kj == 4 * qi + qs),
                            skip_group_check=True)
                if sub >= 0:  # this kj closes q-sub-block `sub`'s bank
                    norm_qsub(b, qi, sub, ot_tiles)

            def norm_qsub(b, qi, qs, ot_tiles):
                ot = ot_tiles[qs // 2][:, qs % 2, :]
                rec = rec_p.tile([128, 2], f32, tag="rc")
                with nc.allow_low_precision(reason="softmax denominators"):
                    nc.vector.reciprocal(out=rec, in_=ot[:, 64::65])
                osb = osb_p.tile([128, 2, 64], f16, tag="ob")
                for h in range(2):
                    nc.vector.tensor_scalar_mul(
                        out=osb[:, h, :],
                        in0=ot[:, h * 65:h * 65 + 64],
                        scalar1=rec[:, h:h + 1])
                if DEBUG and (b, qi) == (0, 0):
                    dsb = osb_p.tile([128, 130], f32, tag="dbg", name="dsb")
                    nc.vector.tensor_copy(out=dsb, in_=ot)
                    nc.sync.dma_start(out=otd[:, qs * 130:(qs + 1) * 130],
                                      in_=dsb)
                    nc.sync.dma_start(
                        out=osd[:, qs * 128:(qs + 1) * 128],
                        in_=osb.rearrange("p a b -> p (a b)"))
                # XBAR transpose needs a CONTIGUOUS destination; strided
                # slices of a big tile produce wrong output on hardware.
                otr = otr_p.tile([128, KCH], f16, tag="otr",
                                 name=f"otr{b}{qi}{qs}")
                nc.sync.dma_start_transpose(out=otr, in_=osb)
                otrs[(b, qi, qs)] = otr

            # ---------------- projection ----------------
            def proj_piece(b, tch, ebp, act_copy=False):
                t0 = b * S + tch * TCH
                pys = _filler_tile()
                for i in range(2):
                    eb = 2 * ebp + i
                    for qs in range(4):
                        nc.tensor.matmul(
                            pys[:, i, qs * KCH:(qs + 1) * KCH],
                            wo_sb[:, eb * 128:(eb + 1) * 128],
                            otrs[(b, tch, qs)],
                            start=True, stop=True, skip_group_check=True)
                ysb = y_p.tile([128, 2, TCH], f16, tag="ysb")
                if act_copy:
                    nc.scalar.activation(
                        out=ysb, in_=pys,
                        func=mybir.ActivationFunctionType.Copy)
                elif SCHED["pool_copies"]:
                    nc.gpsimd.tensor_copy(out=ysb, in_=pys)
                else:
                    nc.vector.tensor_copy(out=ysb, in_=pys)
                eng = nc.gpsimd if SCHED["y_queue"] == "gpsimd" else nc.sync
                # match ysb's (partition, eb-half, col) iteration order on
                # the DRAM side; a flat [256, 512] slice would interleave
                # the output rows pairwise
                yv = yT.rearrange("(a i p) t -> a p i t", i=2, p=128)
                eng.dma_start(out=yv[ebp, :, :, t0:t0 + TCH], in_=ysb)

            # ---------------- schedule ----------------
            # chunks 0,1 up front (ACT does their PSUM->SBUF copies)
            qk_piece(0, 0, True)
            qk_piece(0, 1, True)
            v_piece(0, act_copy=True)
            qk_piece(1, 0, True)
            qk_piece(1, 1, True)
            v_piece(1, act_copy=True)

            # QKV pieces for chunks 2..7 + projection pieces are drained into
            # the 80-block attention stream by credit pacing (total filler PE
            # time / blocks), gated per piece on a readiness block so a
            # not-yet-DMA'd input can't head-of-line-block the in-order PE
            # queue. qi start blocks: (0,*)=0/4/12/24, (1,*)=40/44/52/64.
            QKV_COST = 8 * TCH * 0.4166667
            PROJ_COST = 2 * TCH * 0.4166667
            READY = SCHED["ready"]
            fillers = []  # dicts: cost, ready, fn, chunk?
            for c in range(2, 8):
                for fn in (lambda t=c: qk_piece(t, 0, False),
                           lambda t=c: qk_piece(t, 1, False),
                           lambda t=c: v_piece(t)):
                    fillers.append(
                        {"cost": QKV_COST, "ready": READY[c],
                         "chunk": c, "fn": fn})

            def ensure_chunk(c):
                for f in [f for f in fillers if f.get("chunk") == c]:
                    fillers.remove(f)
                    f["fn"]()

            credit = [SCHED["credit0"]]

            def fill(i):
                while credit[0] > 0:
                    pick = next((f for f in fillers if f["ready"] <= i), None)
                    if pick is None or pick["cost"] > credit[0] + 400:
                        break
                    fillers.remove(pick)
                    pick["fn"]()
                    credit[0] -= pick["cost"]

            # flat block stream, PV delayed one block behind scores/exp so
            # the PE never waits on the current block's exp
            b1o = (3, 2, 1, 0) if SCHED["rev_b1"] else (0, 1, 2, 3)
            stream = [(0, qi, kj) for qi in range(4)
                      for kj in range(4 * qi + 4)]
            stream += [(1, qi, kj) for qi in b1o
                       for kj in range(4 * qi + 4)]
            ots = {}
            pends = []  # [(b, qi, kj, eAB)]

            def flush_pend(limit):
                while len(pends) > limit:
                    pb, pqi, pkj, peAB = pends.pop(0)
                    if (pb, pqi) not in ots:
                        tiles = [
                            psum.tile([128, 2, 130], f32, tag="ot", bufs=2,
                                      name=f"ot{pb}{pqi}{s}")
                            for s in range(2)]
                        for t_ in tiles:
                            nc.vector.memset(t_, 0.0)
                        ots[(pb, pqi)] = tiles

                    pv_block(pb, pqi, pkj, peAB, ots[(pb, pqi)])
                    if pkj == 4 * pqi + 3:
                        last = pb == 1 and pqi <= SCHED.get("act_b1", 0)
                        for ebp in range(4):
                            fillers.append(
                                {"cost": PROJ_COST,
                                 "ready": i_ref[0] + SCHED["proj_lead"],
                                 "fn": lambda b=pb, q=pqi, e=ebp, l=last:
                                 proj_piece(b, q, e, act_copy=(
                                     (SCHED["act_share"] or l)
                                     and e % 2 == 1))})

            i_ref = [0]
            for i, (b, qi, kj) in enumerate(stream):
                i_ref[0] = i
                if kj == 0:
                    if b == 0 and qi >= 2:
                        ensure_chunk(qi)
                    elif b == 1:
                        for c in range(4, 5 + qi):
                            ensure_chunk(c)
                eAB = score_exp(b, qi, kj)
                flush_pend(SCHED["pv_depth"])
                pends.append((b, qi, kj, eAB))
                credit[0] += SCHED["rate"]
                fill(i)
            flush_pend(0)
            for f in fillers:
                f["fn"]()
            if DEBUG:
                nc.sync.dma_start(out=qTd[:, :], in_=qT)
                nc.sync.dma_start(out=kTd[:, :], in_=kT)
                nc.sync.dma_start(
                    out=vd[:, :], in_=vaug.rearrange("p a b c -> p (a b c)"))
                for (db, dqi, dqs), ot_t in otrs.items():
                    qg = db * S + dqi * TCH + dqs * KCH
                    nc.sync.dma_start(out=ocd[:, qg:qg + KCH], in_=ot_t)

    nc.compile()
    return nc


def _host_prep(x, token_positions, w_qkv, w_o):
    """Build per-core input maps."""
    x = np.asarray(x, dtype=np.float32)
    w_qkv = np.asarray(w_qkv, dtype=np.float32)
    w_o = np.asarray(w_o, dtype=np.float32)
    pos = np.asarray(token_positions).astype(np.float64)

    xT = np.ascontiguousarray(x.reshape(T, D).T).astype(np.float16)

    half = DK // 2
    inv_freq = THETA ** (-np.arange(half, dtype=np.float64) / half)  # [32]
    ang = pos[:, None] * inv_freq[None, :]          # [S, 32]
    cos = np.cos(ang).astype(np.float16)            # [S, 32]
    sin = np.sin(ang).astype(np.float16)

    # interleaved pair layout: partition p (within a head's 64) has freq p//2
    cos_rows = np.repeat(cos.T, 2, axis=0)          # [64, S]
    sin_rows = np.repeat(sin.T, 2, axis=0)
    sgn = np.where(np.arange(64) % 2 == 0, -1.0, 1.0).astype(np.float16)
    ssin_rows = sin_rows * sgn[:, None]
    crep = np.vstack([cos_rows, cos_rows])          # [128, 2048]
    ssign = np.vstack([ssin_rows, ssin_rows])

    # strict lower triangle NEG mask for the 128-wide diagonal band, one
    # copy per head: maskb[p, h*128 + j] = NEG if p > j else 0
    jj = np.arange(128)[None, :]
    pp = np.arange(128)[:, None]
    band = np.where(pp > jj, NEG, 0.0).astype(np.float16)
    maskb = np.concatenate([band, band], axis=1)    # [128, 256]

    onesd = np.ones((128, 64), dtype=np.float16)
    identr_np = np.eye(128, dtype=np.float16)

    scale = 1.0 / math.sqrt(DK)
    in_maps = []
    for c in range(NCORES):
        hA, hB = HPC * c, HPC * c + 1
        wq = np.empty((3 * 128, D), dtype=np.float32)
        wq[0:64] = w_qkv[hA * DK:(hA + 1) * DK] * scale
        wq[64:128] = w_qkv[hB * DK:(hB + 1) * DK] * scale
        wq[128:192] = w_qkv[D + hA * DK:D + (hA + 1) * DK]
        wq[192:256] = w_qkv[D + hB * DK:D + (hB + 1) * DK]
        wq[256:320] = w_qkv[2 * D + hA * DK:2 * D + (hA + 1) * DK]
        wq[320:384] = w_qkv[2 * D + hB * DK:2 * D + (hB + 1) * DK]
        wqkvT = np.ascontiguousarray(wq.T).astype(np.float16)

        woTc = np.ascontiguousarray(
            w_o[:, hA * DK:(hB + 1) * DK].T).astype(np.float16)  # [128,1024]

        in_maps.append({
            "xT": xT, "wqkvT": wqkvT, "woT": woTc,
            "crep": crep, "ssign": ssign, "maskb": maskb,
            "onesd": onesd, "identr": identr_np,
        })
    return in_maps


def _get_program():
    global _PROGRAM
    if _PROGRAM is None:
        _PROGRAM = _build_program()
    return _PROGRAM


def run_sharded(in_maps, **kwargs):
    nc = _get_program()
    return run_bass_kernel_spmd(nc, in_maps, core_ids=list(range(NCORES)),
                                **kwargs)


def kernel(x, token_positions, w_qkv, w_o):
    in_maps = _host_prep(x, token_positions, w_qkv, w_o)
    res = run_sharded(in_maps)
    acc = np.zeros((D, T), dtype=np.float64)
    for c in range(NCORES):
        acc += res.results[c]["yT"]
    y = acc.T.astype(np.float32).reshape(B, S, D)
    return y


# revision 62
# speedup vs baseline: 1.1251x; 1.0149x over previous
"""Causal multi-head self-attention (RoPE) Trainium2 kernel.

Model (from the reference nn.Module):
  D_MODEL=1024, NUM_HEADS=16, D_K=64, THETA=10000, BATCH=2, SEQ=2048.
  qkv = x @ w_qkv.T ; q,k get interleaved-pair RoPE; causal softmax(q k^T/8) v;
  out = attn_out @ w_o.T.

Sharding: tensor-parallel over heads. 8 cores x 2 heads each. x is
replicated (transposed on host), per-core w_qkv/w_o head slices. Each core
produces a partial y (full [1024, 4096] f32); host sums partials and
transposes back.

Pipeline per core (all matmul operands f16, PSUM f32):
  - QKV: x resident in SBUF; q/k projected feature-on-partition, RoPE via
    stream_shuffle + fused scalar_tensor_tensor ops (4x DVE mode); V kept
    token-on-partition with an appended ones column per head.
  - Attention: score tiles sT [k=128, q<=512] for both heads in one 2-bank
    PSUM tile; causal mask added on the PE over just the 128-wide diagonal
    band; one exp per k-block on ACT; PV with the exp tile as the
    *stationary* operand -> O accumulates as [q=128, 65*2] (64 dims + the
    softmax denominator per head) using the full 128 output partitions.
  - Normalize: per-partition reciprocal + tensor_scalar, then a DMA
    transpose turns O [q, d] into ocatT [d, q] for the projection.
  - Projection: wo^T . ocatT in 128-row blocks, written straight from PSUM
    to DRAM as f32 by DMA.
  Emission is software-pipelined: QKV chunks of the next batch and
  projection pieces of the previous chunk are interleaved into the
  (ACT-bound) attention block stream so the PE never starves.
"""

import math
import numpy as np
from contextlib import ExitStack

import concourse.bacc as bacc
import concourse.mybir as mybir
import concourse.tile as tile
from concourse.bass_utils import run_bass_kernel_spmd

f32 = mybir.dt.float32
f16 = mybir.dt.float16

D = 1024          # d_model
H = 16            # total heads
DK = 64           # head dim
B = 2
S = 2048
T = B * S         # 4096 tokens
NCORES = 8
HPC = H // NCORES  # heads per core = 2
THETA = 10000.0
NEG = -30000.0     # causal-mask additive constant (exp underflows to 0)

TCH = 512          # token chunk
NTCH = T // TCH    # 8
KCH = 128          # key block
NBLK = T // KCH    # 32

SWAP_MASK = [m ^ 1 for m in range(32)]  # adjacent-pair swap per quadrant

MULT = mybir.AluOpType.mult
ADD = mybir.AluOpType.add

SCHED = {
    "rev_b1": True,        # B1 qi order 3,2,1,0
    "ready": {2: 7, 3: 8, 4: 12, 5: 15, 6: 18, 7: 21},
    "credit0": 0.0,
    "rate": 600.0,
    "proj_lead": 3,
    "act_share": False,    # alternate proj y-copies onto ACT
    "y_queue": "sync",     # which queue triggers y DMAs
    "pool_copies": False,  # y/V PSUM->SBUF copies on gpsimd (Pool)
    "aux_tag": False,      # fillers use their own 1-buf PSUM slot
    "pv_depth": 1,         # blocks of delay between scores/exp and PV
}

DEBUG = False

_PROGRAM = None


def _build_program():
    nc = bacc.Bacc("TRN2", target_bir_lowering=False, debug=False)

    xT = nc.dram_tensor("xT", [D, T], f16, kind="ExternalInput")
    wqkvT = nc.dram_tensor("wqkvT", [D, 3 * 128], f16, kind="ExternalInput")
    woT = nc.dram_tensor("woT", [128, D], f16, kind="ExternalInput")
    crep = nc.dram_tensor("crep", [128, S], f16, kind="ExternalInput")
    ssign = nc.dram_tensor("ssign", [128, S], f16, kind="ExternalInput")
    maskb = nc.dram_tensor("maskb", [128, 2 * 128], f16, kind="ExternalInput")
    identr = nc.dram_tensor("identr", [128, 128], f16, kind="ExternalInput")
    onesd = nc.dram_tensor("onesd", [128, 64], f16, kind="ExternalInput")
    yT = nc.dram_tensor("yT", [D, T], f16, kind="ExternalOutput")
    if DEBUG:
        qTd = nc.dram_tensor("qTd", [128, T], f16, kind="ExternalOutput")
        kTd = nc.dram_tensor("kTd", [128, T], f16, kind="ExternalOutput")
        vd = nc.dram_tensor("vd", [128, NBLK * 130], f16,
                            kind="ExternalOutput")
        ocd = nc.dram_tensor("ocd", [128, T], f16, kind="ExternalOutput")
        ed = nc.dram_tensor("ed", [128, 2 * TCH], f16, kind="ExternalOutput")
        otd = nc.dram_tensor("otd", [128, 4 * 130], f32, kind="ExternalOutput")
        osd = nc.dram_tensor("osd", [128, 4 * 128], f16, kind="ExternalOutput")

    xT_p = xT.rearrange("(n p) t -> p n t", p=128)          # [128, 8, T]
    wq_r = wqkvT.rearrange("(n p) c -> p n c", p=128)       # [128, 8, 384]

    with tile.TileContext(nc) as tc:
        with ExitStack() as ctx:
            singles = ctx.enter_context(tc.tile_pool(name="singles", bufs=1))

            wq_sb = singles.tile([128, 8, 3 * 128], f16)
            wo_sb = singles.tile([128, D], f16)
            crep_sb = singles.tile([128, S], f16)
            ssign_sb = singles.tile([128, S], f16)
            mask_sb = singles.tile([128, 2, 128], f16)
            identr_sb = singles.tile([128, 128], f16)

            # One SP FIFO, ordered by when each tensor is first needed: the
            # DMA_ENGINES device is serial, so arrival order is criticality
            # order. x is resident in SBUF, in 4 groups so early chunks
            # start compute long before the tail groups land. The xcD tail
            # + wo go through the gpsimd SWDGE queue, gated behind xcB by a
            # dummy copy, so the attention-critical transposes on SP slip
            # into the DMA device ahead of them.
            xcA = singles.tile([128, 8, TCH], f16)       # t [0, 512)
            xcB = singles.tile([128, 8, TCH], f16)       # t [512, 1024)
            xcC = singles.tile([128, 8, 2 * TCH], f16)   # t [1024, 2048)
            xcD = singles.tile([128, 8, 4 * TCH], f16)   # t [2048, 4096)
            nc.sync.dma_start(out=wq_sb[:, :, 0:128], in_=wq_r[:, :, 0:128])
            nc.sync.dma_start(out=xcA[:, 0:4, :], in_=xT_p[:, 0:4, 0:512])
            nc.sync.dma_start(out=wq_sb[:, :, 128:256],
                              in_=wq_r[:, :, 128:256])
            nc.sync.dma_start(out=xcA[:, 4:8, :], in_=xT_p[:, 4:8, 0:512])
            nc.sync.dma_start(out=wq_sb[:, :, 256:384],
                              in_=wq_r[:, :, 256:384])
            nc.sync.dma_start(out=crep_sb[:, 0:512], in_=crep[:, 0:512])
            nc.sync.dma_start(out=ssign_sb[:, 0:512], in_=ssign[:, 0:512])
            nc.sync.dma_start(
                out=mask_sb, in_=maskb.rearrange("p (a b) -> p a b", a=2))
            nc.sync.dma_start(out=identr_sb, in_=identr[:, :])
            nc.sync.dma_start(out=xcB, in_=xT_p[:, :, 512:1024])
            nc.sync.dma_start(out=crep_sb[:, 512:2048], in_=crep[:, 512:2048])
            nc.sync.dma_start(out=ssign_sb[:, 512:2048],
                              in_=ssign[:, 512:2048])
            nc.sync.dma_start(out=xcC, in_=xT_p[:, :, 1024:2048])
            nc.sync.dma_start(out=wo_sb, in_=woT[:, :])
            for g in range(4):  # token-quarters: chunk 4+g needs only piece g
                nc.sync.dma_start(
                    out=xcD[:, :, g * 512:(g + 1) * 512],
                    in_=xT_p[:, :, 2048 + g * 512:2048 + (g + 1) * 512])

            def xslice(tch, fo=0, sz=TCH):
                """[128, 8, sz] view of x tokens [tch*512+fo, ...+sz)."""
                t0 = tch * TCH + fo
                if t0 < 512:
                    return xcA[:, :, t0:t0 + sz]
                if t0 < 1024:
                    return xcB[:, :, t0 - 512:t0 - 512 + sz]
                if t0 < 2048:
                    return xcC[:, :, t0 - 1024:t0 - 1024 + sz]
                return xcD[:, :, t0 - 2048:t0 - 2048 + sz]

            qT = singles.tile([128, T], f16)
            kT = singles.tile([128, T], f16)
            # V token-on-partition per 128-token block:
            # [128, blk, head, 65]; col 64 of each head = ones (softmax
            # denominators fall out of the PV matmul's last column).
            vaug = singles.tile([128, NBLK, 2, 65], f16)
            for h in range(2):
                nc.sync.dma_start(out=vaug[:, :, h, 64], in_=onesd[:, 0:NBLK])

            rope = ctx.enter_context(tc.tile_pool(name="rope", bufs=8))
            otr_p = ctx.enter_context(tc.tile_pool(name="otr", bufs=16))
            otrs = {}  # (b, qi, qs) -> contiguous transposed O tile
            eps_p = ctx.enter_context(tc.tile_pool(name="eps", bufs=14))
            osb_p = ctx.enter_context(tc.tile_pool(name="osb", bufs=8))
            rec_p = ctx.enter_context(tc.tile_pool(name="rec", bufs=8))
            y_p = ctx.enter_context(tc.tile_pool(name="yb", bufs=12))
            psum = ctx.enter_context(
                tc.tile_pool(name="ps", bufs=3, space="PSUM"))

            def _filler_tile():
                return psum.tile([128, 2, TCH], f32, tag="sps", bufs=3,
                                 name="aux")

            # ---------------- QKV pieces ----------------
            def rope_emit(ps, dst_sl, s0, act_copy):
                """ps [128,512] f32 PSUM -> RoPE -> dst (f16 SBUF)."""
                psb = rope.tile([128, TCH], f16, tag="psb")
                if act_copy:
                    nc.scalar.activation(
                        out=psb, in_=ps,
                        func=mybir.ActivationFunctionType.Copy)
                else:
                    nc.vector.tensor_copy(out=psb, in_=ps)
                shb = rope.tile([128, TCH], f16, tag="shb")
                nc.vector.stream_shuffle(out=shb, in_=psb, mask=SWAP_MASK)
                t1 = rope.tile([128, TCH], f16, tag="t1")
                nc.vector.tensor_tensor(
                    out=t1, in0=psb, in1=crep_sb[:, s0:s0 + TCH], op=MULT)
                t2 = rope.tile([128, TCH], f16, tag="t2")
                # all-SBUF ops: legal on gpsimd (only PSUM access is not);
                # Pool is otherwise idle, and this unclogs the DVE queue for
                # the norm/evacuation chains
                nc.gpsimd.tensor_tensor(
                    out=t2, in0=shb, in1=ssign_sb[:, s0:s0 + TCH], op=MULT)
                nc.gpsimd.tensor_tensor(out=dst_sl, in0=t1, in1=t2, op=ADD)

            def qk_piece(tch, mb, act_copy):
                """Project q (mb=0) or k (mb=1) for token chunk tch + RoPE."""
                t0 = tch * TCH
                s0 = t0 % S
                big = _filler_tile()
                ps = big[:, 0, :]
                xs = xslice(tch)
                for dc in range(8):
                    nc.tensor.matmul(
                        ps, wq_sb[:, dc, mb * 128:(mb + 1) * 128],
                        xs[:, dc, :],
                        start=(dc == 0), stop=(dc == 7),
                        skip_group_check=True)
                dst = qT if mb == 0 else kT
                rope_emit(ps, dst[:, t0:t0 + TCH], s0, act_copy)

            def v_piece(tch, act_copy=False):
                """V for token chunk tch -> vaug blocks (natural layout)."""
                big = _filler_tile()
                for sub in range(4):
                    blk = tch * 4 + sub
                    pv = big[:, 0, sub * 128:(sub + 1) * 128]
                    xs = xslice(tch, fo=sub * KCH, sz=KCH)
                    for dc in range(8):
                        nc.tensor.matmul(
                            pv, xs[:, dc, :],
                            wq_sb[:, dc, 256:384],
                            start=(dc == 0), stop=(dc == 7),
                            skip_group_check=True)
                    for h in range(2):
                        dst = vaug[:, blk, h, 0:64]
                        srch = pv[:, h * 64:(h + 1) * 64]
                        if act_copy:
                            nc.scalar.activation(
                                out=dst, in_=srch,
                                func=mybir.ActivationFunctionType.Copy)
                        else:
                            nc.vector.tensor_copy(out=dst, in_=srch)

            # ---------------- attention ----------------
            def score_exp(b, qi, kj):
                """Scores + mask + exp for block kj; returns the exp tile."""
                toff = b * S
                q0 = toff + qi * TCH
                k0 = toff + kj * KCH
                sub = kj - 4 * qi
                diag = sub >= 0
                o = max(0, KCH * sub)
                pAB = psum.tile([128, 2, TCH], f32, tag="sps", bufs=3)
                nc.tensor.matmul(
                    pAB[:, 0, o:TCH], kT[0:64, k0:k0 + KCH],
                    qT[0:64, q0 + o:q0 + TCH],
                    start=True, stop=not diag, skip_group_check=True)
                nc.tensor.matmul(
                    pAB[:, 1, o:TCH], kT[64:128, k0:k0 + KCH],
                    qT[64:128, q0 + o:q0 + TCH],
                    start=True, stop=not diag, skip_group_check=True)
                if diag:  # additive causal mask, 128-wide band, both heads
                    nc.tensor.matmul(
                        pAB[:, :, o:o + KCH], identr_sb, mask_sb,
                        start=False, stop=True, skip_group_check=True)
                eAB = eps_p.tile([128, 2, TCH], f16, tag="eT")
                nc.scalar.activation(
                    out=eAB[:, :, o:TCH], in_=pAB[:, :, o:TCH],
                    func=mybir.ActivationFunctionType.Exp)
                if DEBUG and (b, qi, kj) == (0, 0, 0):
                    nc.sync.dma_start(
                        out=ed[:, :], in_=eAB.rearrange("p a b -> p (a b)"))
                return eAB

            def pv_block(b, qi, kj, eAB, ot_tiles):
                """PV matmuls for block kj + norms for completed q-subs."""
                blk = b * 16 + kj
                sub = kj - 4 * qi
                for qs in range(max(0, sub), 4):
                    ot = ot_tiles[qs // 2][:, qs % 2, :]
                    for h in range(2):
                        # start=False always: a start=True from the other
                        # head would re-arm the bank and break this head's
                        # open accumulation (verified on HW); tiles are
                        # memset to 0 instead.
                        nc.tensor.matmul(
                            ot[:, h * 65:(h + 1) * 65],
                            eAB[:, h, qs * KCH:(qs + 1) * KCH],
                            vaug[:, blk, h, :],
                            start=False, stop=(kj == 4 * qi + qs),
                            skip_group_check=True)
                if sub >= 0:  # this kj closes q-sub-block `sub`'s bank
                    norm_qsub(b, qi, sub, ot_tiles)

            def norm_qsub(b, qi, qs, ot_tiles):
                ot = ot_tiles[qs // 2][:, qs % 2, :]
                rec = rec_p.tile([128, 2], f32, tag="rc")
                with nc.allow_low_precision(reason="softmax denominators"):
                    nc.vector.reciprocal(out=rec, in_=ot[:, 64::65])
                osb = osb_p.tile([128, 2, 64], f16, tag="ob")
                for h in range(2):
                    nc.vector.tensor_scalar_mul(
                        out=osb[:, h, :],
                        in0=ot[:, h * 65:h * 65 + 64],
                        scalar1=rec[:, h:h + 1])
                if DEBUG and (b, qi) == (0, 0):
                    dsb = osb_p.tile([128, 130], f32, tag="dbg", name="dsb")
                    nc.vector.tensor_copy(out=dsb, in_=ot)
                    nc.sync.dma_start(out=otd[:, qs * 130:(qs + 1) * 130],
                                      in_=dsb)
                    nc.sync.dma_start(
                        out=osd[:, qs * 128:(qs + 1) * 128],
                        in_=osb.rearrange("p a b -> p (a b)"))
                # XBAR transpose needs a CONTIGUOUS destination; strided
                # slices of a big tile produce wrong output on hardware.
                otr = otr_p.tile([128, KCH], f16, tag="otr",
                                 name=f"otr{b}{qi}{qs}")
                nc.sync.dma_start_transpose(out=otr, in_=osb)
                otrs[(b, qi, qs)] = otr

            # ---------------- projection ----------------
            def proj_piece(b, tch, ebp, act_copy=False):
                t0 = b * S + tch * TCH
                pys = _filler_tile()
                for i in range(2):
                    eb = 2 * ebp + i
                    for qs in range(4):
                        nc.tensor.matmul(
                            pys[:, i, qs * KCH:(qs + 1) * KCH],
                            wo_sb[:, eb * 128:(eb + 1) * 128],
                            otrs[(b, tch, qs)],
                            start=True, stop=True, skip_group_check=True)
                ysb = y_p.tile([128, 2, TCH], f16, tag="ysb")
                if act_copy:
                    nc.scalar.activation(
                        out=ysb, in_=pys,
                        func=mybir.ActivationFunctionType.Copy)
                elif SCHED["pool_copies"]:
                    nc.gpsimd.tensor_copy(out=ysb, in_=pys)
                else:
                    nc.vector.tensor_copy(out=ysb, in_=pys)
                eng = nc.gpsimd if SCHED["y_queue"] == "gpsimd" else nc.sync
                # match ysb's (partition, eb-half, col) iteration order on
                # the DRAM side; a flat [256, 512] slice would interleave
                # the output rows pairwise
                yv = yT.rearrange("(a i p) t -> a p i t", i=2, p=128)
                eng.dma_start(out=yv[ebp, :, :, t0:t0 + TCH], in_=ysb)

            # ---------------- schedule ----------------
            # chunks 0,1 up front (ACT does their PSUM->SBUF copies)
            qk_piece(0, 0, True)
            qk_piece(0, 1, True)
            v_piece(0, act_copy=True)
            qk_piece(1, 0, True)
            qk_piece(1, 1, True)
            v_piece(1, act_copy=True)

            # QKV pieces for chunks 2..7 + projection pieces are drained into
            # the 80-block attention stream by credit pacing (total filler PE
            # time / blocks), gated per piece on a readiness block so a
            # not-yet-DMA'd input can't head-of-line-block the in-order PE
            # queue. qi start blocks: (0,*)=0/4/12/24, (1,*)=40/44/52/64.
            QKV_COST = 8 * TCH * 0.4166667
            PROJ_COST = 2 * TCH * 0.4166667
            READY = SCHED["ready"]
            fillers = []  # dicts: cost, ready, fn, chunk?
            for c in range(2, 8):
                for fn in (lambda t=c: qk_piece(t, 0, False),
                           lambda t=c: qk_piece(t, 1, False),
                           lambda t=c: v_piece(t)):
                    fillers.append(
                        {"cost": QKV_COST, "ready": READY[c],
                         "chunk": c, "fn": fn})

            def ensure_chunk(c):
                for f in [f for f in fillers if f.get("chunk") == c]:
                    fillers.remove(f)
                    f["fn"]()

            credit = [SCHED["credit0"]]

            def fill(i):
                while credit[0] > 0:
                    pick = next((f for f in fillers if f["ready"] <= i), None)
                    if pick is None or pick["cost"] > credit[0] + 400:
                        break
                    fillers.remove(pick)
                    pick["fn"]()
                    credit[0] -= pick["cost"]

            # flat block stream, PV delayed one block behind scores/exp so
            # the PE never waits on the current block's exp
            b1o = (3, 2, 1, 0) if SCHED["rev_b1"] else (0, 1, 2, 3)
            stream = [(0, qi, kj) for qi in range(4)
                      for kj in range(4 * qi + 4)]
            stream += [(1, qi, kj) for qi in b1o
                       for kj in range(4 * qi + 4)]
            ots = {}
            pends = []  # [(b, qi, kj, eAB)]

            def flush_pend(limit):
                while len(pends) > limit:
                    pb, pqi, pkj, peAB = pends.pop(0)
                    if (pb, pqi) not in ots:
                        tiles = [
                            psum.tile([128, 2, 130], f32, tag="ot", bufs=2,
                                      name=f"ot{pb}{pqi}{s}")
                            for s in range(2)]
                        for t_ in tiles:
                            nc.vector.memset(t_, 0.0)
                        ots[(pb, pqi)] = tiles

                    pv_block(pb, pqi, pkj, peAB, ots[(pb, pqi)])
                    if pkj == 4 * pqi + 3:
                        last = pb == 1 and pqi <= SCHED.get("act_b1", 0)
                        for ebp in range(4):
                            fillers.append(
                                {"cost": PROJ_COST,
                                 "ready": i_ref[0] + SCHED["proj_lead"],
                                 "fn": lambda b=pb, q=pqi, e=ebp, l=last:
                                 proj_piece(b, q, e, act_copy=(
                                     (SCHED["act_share"] or l)
                                     and e % 2 == 1))})

            i_ref = [0]
            for i, (b, qi, kj) in enumerate(stream):
                i_ref[0] = i
                if kj == 0:
                    if b == 0 and qi >= 2:
                        ensure_chunk(qi)
                    elif b == 1:
                        for c in range(4, 5 + qi):
                            ensure_chunk(c)
                eAB = score_exp(b, qi, kj)
                flush_pend(SCHED["pv_depth"])
                pends.append((b, qi, kj, eAB))
                credit[0] += SCHED["rate"]
                fill(i)
            flush_pend(0)
            for f in fillers:
                f["fn"]()
            if DEBUG:
                nc.sync.dma_start(out=qTd[:, :], in_=qT)
                nc.sync.dma_start(out=kTd[:, :], in_=kT)
                nc.sync.dma_start(
                    out=vd[:, :], in_=vaug.rearrange("p a b c -> p (a b c)"))
                for (db, dqi, dqs), ot_t in otrs.items():
                    qg = db * S + dqi * TCH + dqs * KCH
                    nc.sync.dma_start(out=ocd[:, qg:qg + KCH], in_=ot_t)

    nc.compile()
    return nc


def _host_prep(x, token_positions, w_qkv, w_o):
    """Build per-core input maps."""
    x = np.asarray(x, dtype=np.float32)
    w_qkv = np.asarray(w_qkv, dtype=np.float32)
    w_o = np.asarray(w_o, dtype=np.float32)
    pos = np.asarray(token_positions).astype(np.float64)

    xT = np.ascontiguousarray(x.reshape(T, D).T).astype(np.float16)

    half = DK // 2
    inv_freq = THETA ** (-np.arange(half, dtype=np.float64) / half)  # [32]
    ang = pos[:, None] * inv_freq[None, :]          # [S, 32]
    cos = np.cos(ang).astype(np.float16)            # [S, 32]
    sin = np.sin(ang).astype(np.float16)

    # interleaved pair layout: partition p (within a head's 64) has freq p//2
    cos_rows = np.repeat(cos.T, 2, axis=0)          # [64, S]
    sin_rows = np.repeat(sin.T, 2, axis=0)
    sgn = np.where(np.arange(64) % 2 == 0, -1.0, 1.0).astype(np.float16)
    ssin_rows = sin_rows * sgn[:, None]
    crep = np.vstack([cos_rows, cos_rows])          # [128, 2048]
    ssign = np.vstack([ssin_rows, ssin_rows])

    # strict lower triangle NEG mask for the 128-wide diagonal band, one
    # copy per head: maskb[p, h*128 + j] = NEG if p > j else 0
    jj = np.arange(128)[None, :]
    pp = np.arange(128)[:, None]
    band = np.where(pp > jj, NEG, 0.0).astype(np.float16)
    maskb = np.concatenate([band, band], axis=1)    # [128, 256]

    onesd = np.ones((128, 64), dtype=np.float16)
    identr_np = np.eye(128, dtype=np.float16)

    scale = 1.0 / math.sqrt(DK)
    in_maps = []
    for c in range(NCORES):
        hA, hB = HPC * c, HPC * c + 1
        wq = np.empty((3 * 128, D), dtype=np.float32)
        wq[0:64] = w_qkv[hA * DK:(hA + 1) * DK] * scale
        wq[64:128] = w_qkv[hB * DK:(hB + 1) * DK] * scale
        wq[128:192] = w_qkv[D + hA * DK:D + (hA + 1) * DK]
        wq[192:256] = w_qkv[D + hB * DK:D + (hB + 1) * DK]
        wq[256:320] = w_qkv[2 * D + hA * DK:2 * D + (hA + 1) * DK]
        wq[320:384] = w_qkv[2 * D + hB * DK:2 * D + (hB + 1) * DK]
        wqkvT = np.ascontiguousarray(wq.T).astype(np.float16)

        woTc = np.ascontiguousarray(
            w_o[:, hA * DK:(hB + 1) * DK].T).astype(np.float16)  # [128,1024]

        in_maps.append({
            "xT": xT, "wqkvT": wqkvT, "woT": woTc,
            "crep": crep, "ssign": ssign, "maskb": maskb,
            "onesd": onesd, "identr": identr_np,
        })
    return in_maps


def _get_program():
    global _PROGRAM
    if _PROGRAM is None:
        _PROGRAM = _build_program()
    return _PROGRAM


def run_sharded(in_maps, **kwargs):
    nc = _get_program()
    return run_bass_kernel_spmd(nc, in_maps, core_ids=list(range(NCORES)),
                                **kwargs)


def kernel(x, token_positions, w_qkv, w_o):
    in_maps = _host_prep(x, token_positions, w_qkv, w_o)
    res = run_sharded(in_maps)
    acc = np.zeros((D, T), dtype=np.float64)
    for c in range(NCORES):
        acc += res.results[c]["yT"]
    y = acc.T.astype(np.float32).reshape(B, S, D)
    return y


# revision 63
# speedup vs baseline: 1.1364x; 1.0100x over previous
"""Causal multi-head self-attention (RoPE) Trainium2 kernel.

Model (from the reference nn.Module):
  D_MODEL=1024, NUM_HEADS=16, D_K=64, THETA=10000, BATCH=2, SEQ=2048.
  qkv = x @ w_qkv.T ; q,k get interleaved-pair RoPE; causal softmax(q k^T/8) v;
  out = attn_out @ w_o.T.

Sharding: tensor-parallel over heads. 8 cores x 2 heads each. x is
replicated (transposed on host), per-core w_qkv/w_o head slices. Each core
produces a partial y (full [1024, 4096] f32); host sums partials and
transposes back.

Pipeline per core (all matmul operands f16, PSUM f32):
  - QKV: x resident in SBUF; q/k projected feature-on-partition, RoPE via
    stream_shuffle + fused scalar_tensor_tensor ops (4x DVE mode); V kept
    token-on-partition with an appended ones column per head.
  - Attention: score tiles sT [k=128, q<=512] for both heads in one 2-bank
    PSUM tile; causal mask added on the PE over just the 128-wide diagonal
    band; one exp per k-block on ACT; PV with the exp tile as the
    *stationary* operand -> O accumulates as [q=128, 65*2] (64 dims + the
    softmax denominator per head) using the full 128 output partitions.
  - Normalize: per-partition reciprocal + tensor_scalar, then a DMA
    transpose turns O [q, d] into ocatT [d, q] for the projection.
  - Projection: wo^T . ocatT in 128-row blocks, written straight from PSUM
    to DRAM as f32 by DMA.
  Emission is software-pipelined: QKV chunks of the next batch and
  projection pieces of the previous chunk are interleaved into the
  (ACT-bound) attention block stream so the PE never starves.
"""

import math
import numpy as np
from contextlib import ExitStack

import concourse.bacc as bacc
import concourse.mybir as mybir
import concourse.tile as tile
from concourse.bass_utils import run_bass_kernel_spmd

f32 = mybir.dt.float32
f16 = mybir.dt.float16

D = 1024          # d_model
H = 16            # total heads
DK = 64           # head dim
B = 2
S = 2048
T = B * S         # 4096 tokens
NCORES = 8
HPC = H // NCORES  # heads per core = 2
THETA = 10000.0
NEG = -30000.0     # causal-mask additive constant (exp underflows to 0)

TCH = 512          # token chunk
NTCH = T // TCH    # 8
KCH = 128          # key block
NBLK = T // KCH    # 32

SWAP_MASK = [m ^ 1 for m in range(32)]  # adjacent-pair swap per quadrant

MULT = mybir.AluOpType.mult
ADD = mybir.AluOpType.add

SCHED = {
    "rev_b1": True,        # B1 qi order 3,2,1,0
    "ready": {2: 7, 3: 8, 4: 12, 5: 15, 6: 18, 7: 21},
    "credit0": 0.0,
    "rate": 640.0,
    "proj_lead": 3,
    "act_share": False,    # alternate proj y-copies onto ACT
    "y_queue": "sync",     # which queue triggers y DMAs
    "pool_copies": False,  # y/V PSUM->SBUF copies on gpsimd (Pool)
    "aux_tag": False,      # fillers use their own 1-buf PSUM slot
    "pv_depth": 1,         # blocks of delay between scores/exp and PV
}

DEBUG = False

_PROGRAM = None


def _build_program():
    nc = bacc.Bacc("TRN2", target_bir_lowering=False, debug=False)

    xT = nc.dram_tensor("xT", [D, T], f16, kind="ExternalInput")
    wqkvT = nc.dram_tensor("wqkvT", [D, 3 * 128], f16, kind="ExternalInput")
    woT = nc.dram_tensor("woT", [128, D], f16, kind="ExternalInput")
    crep = nc.dram_tensor("crep", [128, S], f16, kind="ExternalInput")
    ssign = nc.dram_tensor("ssign", [128, S], f16, kind="ExternalInput")
    maskb = nc.dram_tensor("maskb", [128, 2 * 128], f16, kind="ExternalInput")
    identr = nc.dram_tensor("identr", [128, 128], f16, kind="ExternalInput")
    onesd = nc.dram_tensor("onesd", [128, 64], f16, kind="ExternalInput")
    yT = nc.dram_tensor("yT", [D, T], f16, kind="ExternalOutput")
    if DEBUG:
        qTd = nc.dram_tensor("qTd", [128, T], f16, kind="ExternalOutput")
        kTd = nc.dram_tensor("kTd", [128, T], f16, kind="ExternalOutput")
        vd = nc.dram_tensor("vd", [128, NBLK * 130], f16,
                            kind="ExternalOutput")
        ocd = nc.dram_tensor("ocd", [128, T], f16, kind="ExternalOutput")
        ed = nc.dram_tensor("ed", [128, 2 * TCH], f16, kind="ExternalOutput")
        otd = nc.dram_tensor("otd", [128, 4 * 130], f32, kind="ExternalOutput")
        osd = nc.dram_tensor("osd", [128, 4 * 128], f16, kind="ExternalOutput")

    xT_p = xT.rearrange("(n p) t -> p n t", p=128)          # [128, 8, T]
    wq_r = wqkvT.rearrange("(n p) c -> p n c", p=128)       # [128, 8, 384]

    with tile.TileContext(nc) as tc:
        with ExitStack() as ctx:
            singles = ctx.enter_context(tc.tile_pool(name="singles", bufs=1))

            wq_sb = singles.tile([128, 8, 3 * 128], f16)
            wo_sb = singles.tile([128, D], f16)
            crep_sb = singles.tile([128, S], f16)
            ssign_sb = singles.tile([128, S], f16)
            mask_sb = singles.tile([128, 2, 128], f16)
            identr_sb = singles.tile([128, 128], f16)

            # One SP FIFO, ordered by when each tensor is first needed: the
            # DMA_ENGINES device is serial, so arrival order is criticality
            # order. x is resident in SBUF, in 4 groups so early chunks
            # start compute long before the tail groups land. The xcD tail
            # + wo go through the gpsimd SWDGE queue, gated behind xcB by a
            # dummy copy, so the attention-critical transposes on SP slip
            # into the DMA device ahead of them.
            xcA = singles.tile([128, 8, TCH], f16)       # t [0, 512)
            xcB = singles.tile([128, 8, TCH], f16)       # t [512, 1024)
            xcC = singles.tile([128, 8, 2 * TCH], f16)   # t [1024, 2048)
            xcD = singles.tile([128, 8, 4 * TCH], f16)   # t [2048, 4096)
            nc.sync.dma_start(out=wq_sb[:, :, 0:128], in_=wq_r[:, :, 0:128])
            nc.sync.dma_start(out=xcA[:, 0:4, :], in_=xT_p[:, 0:4, 0:512])
            nc.sync.dma_start(out=wq_sb[:, :, 128:256],
                              in_=wq_r[:, :, 128:256])
            nc.sync.dma_start(out=xcA[:, 4:8, :], in_=xT_p[:, 4:8, 0:512])
            nc.sync.dma_start(out=wq_sb[:, :, 256:384],
                              in_=wq_r[:, :, 256:384])
            nc.sync.dma_start(out=crep_sb[:, 0:512], in_=crep[:, 0:512])
            nc.sync.dma_start(out=ssign_sb[:, 0:512], in_=ssign[:, 0:512])
            nc.sync.dma_start(
                out=mask_sb, in_=maskb.rearrange("p (a b) -> p a b", a=2))
            nc.sync.dma_start(out=identr_sb, in_=identr[:, :])
            nc.sync.dma_start(out=xcB, in_=xT_p[:, :, 512:1024])
            nc.sync.dma_start(out=crep_sb[:, 512:2048], in_=crep[:, 512:2048])
            nc.sync.dma_start(out=ssign_sb[:, 512:2048],
                              in_=ssign[:, 512:2048])
            nc.sync.dma_start(out=xcC, in_=xT_p[:, :, 1024:2048])
            nc.sync.dma_start(out=wo_sb, in_=woT[:, :])
            for g in range(4):  # token-quarters: chunk 4+g needs only piece g
                nc.sync.dma_start(
                    out=xcD[:, :, g * 512:(g + 1) * 512],
                    in_=xT_p[:, :, 2048 + g * 512:2048 + (g + 1) * 512])

            def xslice(tch, fo=0, sz=TCH):
                """[128, 8, sz] view of x tokens [tch*512+fo, ...+sz)."""
                t0 = tch * TCH + fo
                if t0 < 512:
                    return xcA[:, :, t0:t0 + sz]
                if t0 < 1024:
                    return xcB[:, :, t0 - 512:t0 - 512 + sz]
                if t0 < 2048:
                    return xcC[:, :, t0 - 1024:t0 - 1024 + sz]
                return xcD[:, :, t0 - 2048:t0 - 2048 + sz]

            qT = singles.tile([128, T], f16)
            kT = singles.tile([128, T], f16)
            # V token-on-partition per 128-token block:
            # [128, blk, head, 65]; col 64 of each head = ones (softmax
            # denominators fall out of the PV matmul's last column).
            vaug = singles.tile([128, NBLK, 2, 65], f16)
            for h in range(2):
                nc.sync.dma_start(out=vaug[:, :, h, 64], in_=onesd[:, 0:NBLK])

            rope = ctx.enter_context(tc.tile_pool(name="rope", bufs=8))
            otr_p = ctx.enter_context(tc.tile_pool(name="otr", bufs=16))
            otrs = {}  # (b, qi, qs) -> contiguous transposed O tile
            eps_p = ctx.enter_context(tc.tile_pool(name="eps", bufs=14))
            osb_p = ctx.enter_context(tc.tile_pool(name="osb", bufs=8))
            rec_p = ctx.enter_context(tc.tile_pool(name="rec", bufs=8))
            y_p = ctx.enter_context(tc.tile_pool(name="yb", bufs=12))
            psum = ctx.enter_context(
                tc.tile_pool(name="ps", bufs=3, space="PSUM"))

            def _filler_tile():
                return psum.tile([128, 2, TCH], f32, tag="sps", bufs=3,
                                 name="aux")

            # ---------------- QKV pieces ----------------
            def rope_emit(ps, dst_sl, s0, act_copy):
                """ps [128,512] f32 PSUM -> RoPE -> dst (f16 SBUF)."""
                psb = rope.tile([128, TCH], f16, tag="psb")
                if act_copy:
                    nc.scalar.activation(
                        out=psb, in_=ps,
                        func=mybir.ActivationFunctionType.Copy)
                else:
                    nc.vector.tensor_copy(out=psb, in_=ps)
                shb = rope.tile([128, TCH], f16, tag="shb")
                nc.vector.stream_shuffle(out=shb, in_=psb, mask=SWAP_MASK)
                t1 = rope.tile([128, TCH], f16, tag="t1")
                nc.vector.tensor_tensor(
                    out=t1, in0=psb, in1=crep_sb[:, s0:s0 + TCH], op=MULT)
                t2 = rope.tile([128, TCH], f16, tag="t2")
                # all-SBUF ops: legal on gpsimd (only PSUM access is not);
                # Pool is otherwise idle, and this unclogs the DVE queue for
                # the norm/evacuation chains
                nc.gpsimd.tensor_tensor(
                    out=t2, in0=shb, in1=ssign_sb[:, s0:s0 + TCH], op=MULT)
                nc.gpsimd.tensor_tensor(out=dst_sl, in0=t1, in1=t2, op=ADD)

            def qk_piece(tch, mb, act_copy):
                """Project q (mb=0) or k (mb=1) for token chunk tch + RoPE."""
                t0 = tch * TCH
                s0 = t0 % S
                big = _filler_tile()
                ps = big[:, 0, :]
                xs = xslice(tch)
                for dc in range(8):
                    nc.tensor.matmul(
                        ps, wq_sb[:, dc, mb * 128:(mb + 1) * 128],
                        xs[:, dc, :],
                        start=(dc == 0), stop=(dc == 7),
                        skip_group_check=True)
                dst = qT if mb == 0 else kT
                rope_emit(ps, dst[:, t0:t0 + TCH], s0, act_copy)

            def v_piece(tch, act_copy=False):
                """V for token chunk tch -> vaug blocks (natural layout)."""
                big = _filler_tile()
                for sub in range(4):
                    blk = tch * 4 + sub
                    pv = big[:, 0, sub * 128:(sub + 1) * 128]
                    xs = xslice(tch, fo=sub * KCH, sz=KCH)
                    for dc in range(8):
                        nc.tensor.matmul(
                            pv, xs[:, dc, :],
                            wq_sb[:, dc, 256:384],
                            start=(dc == 0), stop=(dc == 7),
                            skip_group_check=True)
                    for h in range(2):
                        dst = vaug[:, blk, h, 0:64]
                        srch = pv[:, h * 64:(h + 1) * 64]
                        if act_copy:
                            nc.scalar.activation(
                                out=dst, in_=srch,
                                func=mybir.ActivationFunctionType.Copy)
                        else:
                            nc.vector.tensor_copy(out=dst, in_=srch)

            # ---------------- attention ----------------
            def score_exp(b, qi, kj):
                """Scores + mask + exp for block kj; returns the exp tile."""
                toff = b * S
                q0 = toff + qi * TCH
                k0 = toff + kj * KCH
                sub = kj - 4 * qi
                diag = sub >= 0
                o = max(0, KCH * sub)
                pAB = psum.tile([128, 2, TCH], f32, tag="sps", bufs=3)
                nc.tensor.matmul(
                    pAB[:, 0, o:TCH], kT[0:64, k0:k0 + KCH],
                    qT[0:64, q0 + o:q0 + TCH],
                    start=True, stop=not diag, skip_group_check=True)
                nc.tensor.matmul(
                    pAB[:, 1, o:TCH], kT[64:128, k0:k0 + KCH],
                    qT[64:128, q0 + o:q0 + TCH],
                    start=True, stop=not diag, skip_group_check=True)
                if diag:  # additive causal mask, 128-wide band, both heads
                    nc.tensor.matmul(
                        pAB[:, :, o:o + KCH], identr_sb, mask_sb,
                        start=False, stop=True, skip_group_check=True)
                eAB = eps_p.tile([128, 2, TCH], f16, tag="eT")
                nc.scalar.activation(
                    out=eAB[:, :, o:TCH], in_=pAB[:, :, o:TCH],
                    func=mybir.ActivationFunctionType.Exp)
                if DEBUG and (b, qi, kj) == (0, 0, 0):
                    nc.sync.dma_start(
                        out=ed[:, :], in_=eAB.rearrange("p a b -> p (a b)"))
                return eAB

            def pv_block(b, qi, kj, eAB, ot_tiles):
                """PV matmuls for block kj + norms for completed q-subs."""
                blk = b * 16 + kj
                sub = kj - 4 * qi
                for qs in range(max(0, sub), 4):
                    ot = ot_tiles[qs // 2][:, qs % 2, :]
                    for h in range(2):
                        # start=False always: a start=True from the other
                        # head would re-arm the bank and break this head's
                        # open accumulation (verified on HW); tiles are
                        # memset to 0 instead.
                        nc.tensor.matmul(
                            ot[:, h * 65:(h + 1) * 65],
                            eAB[:, h, qs * KCH:(qs + 1) * KCH],
                            vaug[:, blk, h, :],
                            start=False, stop=(kj == 4 * qi + qs),
                            skip_group_check=True)
                if sub >= 0:  # this kj closes q-sub-block `sub`'s bank
                    norm_qsub(b, qi, sub, ot_tiles)

            def norm_qsub(b, qi, qs, ot_tiles):
                ot = ot_tiles[qs // 2][:, qs % 2, :]
                rec = rec_p.tile([128, 2], f32, tag="rc")
                with nc.allow_low_precision(reason="softmax denominators"):
                    nc.vector.reciprocal(out=rec, in_=ot[:, 64::65])
                osb = osb_p.tile([128, 2, 64], f16, tag="ob")
                for h in range(2):
                    nc.vector.tensor_scalar_mul(
                        out=osb[:, h, :],
                        in0=ot[:, h * 65:h * 65 + 64],
                        scalar1=rec[:, h:h + 1])
                if DEBUG and (b, qi) == (0, 0):
                    dsb = osb_p.tile([128, 130], f32, tag="dbg", name="dsb")
                    nc.vector.tensor_copy(out=dsb, in_=ot)
                    nc.sync.dma_start(out=otd[:, qs * 130:(qs + 1) * 130],
                                      in_=dsb)
                    nc.sync.dma_start(
                        out=osd[:, qs * 128:(qs + 1) * 128],
                        in_=osb.rearrange("p a b -> p (a b)"))
                # XBAR transpose needs a CONTIGUOUS destination; strided
                # slices of a big tile produce wrong output on hardware.
                otr = otr_p.tile([128, KCH], f16, tag="otr",
                                 name=f"otr{b}{qi}{qs}")
                nc.sync.dma_start_transpose(out=otr, in_=osb)
                otrs[(b, qi, qs)] = otr

            # ---------------- projection ----------------
            def proj_piece(b, tch, ebp, act_copy=False):
                t0 = b * S + tch * TCH
                pys = _filler_tile()
                for i in range(2):
                    eb = 2 * ebp + i
                    for qs in range(4):
                        nc.tensor.matmul(
                            pys[:, i, qs * KCH:(qs + 1) * KCH],
                            wo_sb[:, eb * 128:(eb + 1) * 128],
                            otrs[(b, tch, qs)],
                            start=True, stop=True, skip_group_check=True)
                ysb = y_p.tile([128, 2, TCH], f16, tag="ysb")
                if act_copy:
                    nc.scalar.activation(
                        out=ysb, in_=pys,
                        func=mybir.ActivationFunctionType.Copy)
                elif SCHED["pool_copies"]:
                    nc.gpsimd.tensor_copy(out=ysb, in_=pys)
                else:
                    nc.vector.tensor_copy(out=ysb, in_=pys)
                eng = nc.gpsimd if SCHED["y_queue"] == "gpsimd" else nc.sync
                # match ysb's (partition, eb-half, col) iteration order on
                # the DRAM side; a flat [256, 512] slice would interleave
                # the output rows pairwise
                yv = yT.rearrange("(a i p) t -> a p i t", i=2, p=128)
                eng.dma_start(out=yv[ebp, :, :, t0:t0 + TCH], in_=ysb)

            # ---------------- schedule ----------------
            # chunks 0,1 up front (ACT does their PSUM->SBUF copies)
            qk_piece(0, 0, True)
            qk_piece(0, 1, True)
            v_piece(0, act_copy=True)
            qk_piece(1, 0, True)
            qk_piece(1, 1, True)
            v_piece(1, act_copy=True)

            # QKV pieces for chunks 2..7 + projection pieces are drained into
            # the 80-block attention stream by credit pacing (total filler PE
            # time / blocks), gated per piece on a readiness block so a
            # not-yet-DMA'd input can't head-of-line-block the in-order PE
            # queue. qi start blocks: (0,*)=0/4/12/24, (1,*)=40/44/52/64.
            QKV_COST = 8 * TCH * 0.4166667
            PROJ_COST = 2 * TCH * 0.4166667
            READY = SCHED["ready"]
            fillers = []  # dicts: cost, ready, fn, chunk?
            for c in range(2, 8):
                for fn in (lambda t=c: qk_piece(t, 0, False),
                           lambda t=c: qk_piece(t, 1, False),
                           lambda t=c: v_piece(t)):
                    fillers.append(
                        {"cost": QKV_COST, "ready": READY[c],
                         "chunk": c, "fn": fn})

            def ensure_chunk(c):
                for f in [f for f in fillers if f.get("chunk") == c]:
                    fillers.remove(f)
                    f["fn"]()

            credit = [SCHED["credit0"]]

            def fill(i):
                while credit[0] > 0:
                    pick = next((f for f in fillers if f["ready"] <= i), None)
                    if pick is None or pick["cost"] > credit[0] + 400:
                        break
                    fillers.remove(pick)
                    pick["fn"]()
                    credit[0] -= pick["cost"]

            # flat block stream, PV delayed one block behind scores/exp so
            # the PE never waits on the current block's exp
            b1o = (3, 2, 1, 0) if SCHED["rev_b1"] else (0, 1, 2, 3)
            stream = [(0, qi, kj) for qi in range(4)
                      for kj in range(4 * qi + 4)]
            stream += [(1, qi, kj) for qi in b1o
                       for kj in range(4 * qi + 4)]
            ots = {}
            pends = []  # [(b, qi, kj, eAB)]

            def flush_pend(limit):
                while len(pends) > limit:
                    pb, pqi, pkj, peAB = pends.pop(0)
                    if (pb, pqi) not in ots:
                        tiles = [
                            psum.tile([128, 2, 130], f32, tag="ot", bufs=2,
                                      name=f"ot{pb}{pqi}{s}")
                            for s in range(2)]
                        for t_ in tiles:
                            nc.vector.memset(t_, 0.0)
                        ots[(pb, pqi)] = tiles

                    pv_block(pb, pqi, pkj, peAB, ots[(pb, pqi)])
                    if pkj == 4 * pqi + 3:
                        last = pb == 1 and pqi <= SCHED.get("act_b1", 0)
                        for ebp in range(4):
                            fillers.append(
                                {"cost": PROJ_COST,
                                 "ready": i_ref[0] + SCHED["proj_lead"],
                                 "fn": lambda b=pb, q=pqi, e=ebp, l=last:
                                 proj_piece(b, q, e, act_copy=(
                                     (SCHED["act_share"] or l)
                                     and e % 2 == 1))})

            i_ref = [0]
            for i, (b, qi, kj) in enumerate(stream):
                i_ref[0] = i
                if kj == 0:
                    if b == 0 and qi >= 2:
                        ensure_chunk(qi)
                    elif b == 1:
                        for c in range(4, 5 + qi):
                            ensure_chunk(c)
                eAB = score_exp(b, qi, kj)
                flush_pend(SCHED["pv_depth"])
                pends.append((b, qi, kj, eAB))
                credit[0] += SCHED["rate"]
                fill(i)
            flush_pend(0)
            for f in fillers:
                f["fn"]()
            if DEBUG:
                nc.sync.dma_start(out=qTd[:, :], in_=qT)
                nc.sync.dma_start(out=kTd[:, :], in_=kT)
                nc.sync.dma_start(
                    out=vd[:, :], in_=vaug.rearrange("p a b c -> p (a b c)"))
                for (db, dqi, dqs), ot_t in otrs.items():
                    qg = db * S + dqi * TCH + dqs * KCH
                    nc.sync.dma_start(out=ocd[:, qg:qg + KCH], in_=ot_t)

    nc.compile()
    return nc


def _host_prep(x, token_positions, w_qkv, w_o):
    """Build per-core input maps."""
    x = np.asarray(x, dtype=np.float32)
    w_qkv = np.asarray(w_qkv, dtype=np.float32)
    w_o = np.asarray(w_o, dtype=np.float32)
    pos = np.asarray(token_positions).astype(np.float64)

    xT = np.ascontiguousarray(x.reshape(T, D).T).astype(np.float16)

    half = DK // 2
    inv_freq = THETA ** (-np.arange(half, dtype=np.float64) / half)  # [32]
    ang = pos[:, None] * inv_freq[None, :]          # [S, 32]
    cos = np.cos(ang).astype(np.float16)            # [S, 32]
    sin = np.sin(ang).astype(np.float16)

    # interleaved pair layout: partition p (within a head's 64) has freq p//2
    cos_rows = np.repeat(cos.T, 2, axis=0)          # [64, S]
    sin_rows = np.repeat(sin.T, 2, axis=0)
    sgn = np.where(np.arange(64) % 2 == 0, -1.0, 1.0).astype(np.float16)
    ssin_rows = sin_rows * sgn[:, None]
    crep = np.vstack([cos_rows, cos_rows])          # [128, 2048]
    ssign = np.vstack([ssin_rows, ssin_rows])

    # strict lower triangle NEG mask for the 128-wide diagonal band, one
    # copy per head: maskb[p, h*128 + j] = NEG if p > j else 0
    jj = np.arange(128)[None, :]
    pp = np.arange(128)[:, None]
    band = np.where(pp > jj, NEG, 0.0).astype(np.float16)
    maskb = np.concatenate([band, band], axis=1)    # [128, 256]

    onesd = np.ones((128, 64), dtype=np.float16)
    identr_np = np.eye(128, dtype=np.float16)

    scale = 1.0 / math.sqrt(DK)
    in_maps = []
    for c in range(NCORES):
        hA, hB = HPC * c, HPC * c + 1
        wq = np.empty((3 * 128, D), dtype=np.float32)
        wq[0:64] = w_qkv[hA * DK:(hA + 1) * DK] * scale
        wq[64:128] = w_qkv[hB * DK:(hB + 1) * DK] * scale
        wq[128:192] = w_qkv[D + hA * DK:D + (hA + 1) * DK]
        wq[192:256] = w_qkv[D + hB * DK:D + (hB + 1) * DK]
        wq[256:320] = w_qkv[2 * D + hA * DK:2 * D + (hA + 1) * DK]
        wq[320:384] = w_qkv[2 * D + hB * DK:2 * D + (hB + 1) * DK]
        wqkvT = np.ascontiguousarray(wq.T).astype(np.float16)

        woTc = np.ascontiguousarray(
            w_o[:, hA * DK:(hB + 1) * DK].T).astype(np.float16)  # [128,1024]

        in_maps.append({
            "xT": xT, "wqkvT": wqkvT, "woT": woTc,
            "crep": crep, "ssign": ssign, "maskb": maskb,
            "onesd": onesd, "identr": identr_np,
        })
    return in_maps


def _get_program():
    global _PROGRAM
    if _PROGRAM is None:
        _PROGRAM = _build_program()
    return _PROGRAM


def run_sharded(in_maps, **kwargs):
    nc = _get_program()
    return run_bass_kernel_spmd(nc, in_maps, core_ids=list(range(NCORES)),
                                **kwargs)


def kernel(x, token_positions, w_qkv, w_o):
    in_maps = _host_prep(x, token_positions, w_qkv, w_o)
    res = run_sharded(in_maps)
    acc = np.zeros((D, T), dtype=np.float64)
    for c in range(NCORES):
        acc += res.results[c]["yT"]
    y = acc.T.astype(np.float32).reshape(B, S, D)
    return y
